# revision 7
# baseline (speedup 1.0000x reference)
"""Trainium2 Bass kernel for nn_DDLTransformerBlock (8 NeuronCores).

Sharding: core g in 0..7 -> batch b=g//4, heads [4*(g%4), 4*(g%4)+4) for the
attention part (feature-major, fp32r matmuls, split-x1/x2 rope layout, rk
folded into the exp scale); one 8-core AllToAll redistributes attention
output head-blocks to token owners (each core owns a 256-token slice of
EACH batch); O-projection, delta-residual, SwiGLU MLP and the second delta
run token-sharded with fully replicated weights.
"""
import ml_dtypes
import numpy as np
import concourse.bacc as bacc
import concourse.mybir as mybir
from concourse.tile import TileContext
from concourse.bass_utils import run_bass_kernel_spmd

B, T, D, H, HD, FH = 2, 2048, 1024, 16, 64, 2752
FHP = 2816           # FH padded to 22*128
NKT = D // 128       # 8 k-tiles
NFT = FHP // 128     # 22
NC = 8
MASKVAL = -1e6
F32 = mybir.dt.float32
F32R = mybir.dt.float32r
BF16 = mybir.dt.bfloat16
AF = mybir.ActivationFunctionType

# consts blob column layout (fp32r, [128, CCOLS])
C_I = 0            # identity 128
C_H4X32 = 128      # 128
C_H4 = 256         # 4
C_ONES = 260       # 1
C_MASK = 261       # 4*512
C_GW1 = C_MASK + 4 * 512   # 8
C_WV1 = C_GW1 + 8          # 8
C_GW2 = C_WV1 + 8          # 8
C_WV2 = C_GW2 + 8          # 8
C_SEL = C_WV2 + 8          # 8*128 (sel_kt: [16,128] blocks)
C_ONES8 = C_SEL + 8 * 128  # 8 (all-ones cols)
C_H4P8 = C_ONES8 + 8       # 8 (H4 padded to 8 cols)
CCOLS = C_H4P8 + 8
# eps tile columns (f32, [128, 8])
E_1EM6, E_RK, E_CS, E_RT, E_G1B, E_G2B, E_ZERO, E_TWO = range(8)

_CACHE = {}


def _build_nc():
    nc = bacc.Bacc("TRN2", target_bir_lowering=False, num_devices=NC)
    xT_in = nc.declare_dram_parameter("xT", [D, T], F32R, isOutput=False)
    xo_in = nc.declare_dram_parameter("x_own", [D, 512], F32R, isOutput=False)
    wq_in = nc.declare_dram_parameter("wq_sb", [D, 256], F32R, isOutput=False)
    wk_in = nc.declare_dram_parameter("wk_sb", [D, 256], F32R, isOutput=False)
    wv_in = nc.declare_dram_parameter("wv_sb", [D, 256], F32R, isOutput=False)
    wo_in = nc.declare_dram_parameter("wo_sb", [D, D], F32R, isOutput=False)
    w1_in = nc.declare_dram_parameter("w1_sb", [D, FHP], F32R, isOutput=False)
    w3_in = nc.declare_dram_parameter("w3_sb", [D, FHP], F32R, isOutput=False)
    w2_in = nc.declare_dram_parameter("w2_sb", [FHP, D], BF16, isOutput=False)
    cn_in = nc.declare_dram_parameter("consts", [128, CCOLS], F32R, isOutput=False)
    tb_in = nc.declare_dram_parameter("tabs", [128, 2 * T], F32, isOutput=False)
    ep_in = nc.declare_dram_parameter("eps", [128, 8], F32, isOutput=False)
    out_d = nc.declare_dram_parameter("x_out", [D, 512], F32, isOutput=True)

    def r3(dram, p=128):
        # view [R, C] dram as (p, ktile, C) for tile DMAs
        return dram[:].rearrange("(k p) t -> p k t", p=p)

    with TileContext(nc) as tc:
        with tc.tile_pool(name="glob", bufs=1) as glob, \
             tc.tile_pool(name="dram", bufs=1, space="DRAM") as dram:
            consts = glob.tile([128, CCOLS], F32R)
            nc.sync.dma_start(out=consts[:], in_=cn_in[:])
            tabs = glob.tile([128, 2 * T], F32)
            nc.sync.dma_start(out=tabs[:], in_=tb_in[:])
            eps = glob.tile([128, 8], F32)
            nc.sync.dma_start(out=eps[:], in_=ep_in[:])
            xown = glob.tile([128, NKT, 512], F32R)
            nc.sync.dma_start(out=xown[:], in_=r3(xo_in))

            I128 = consts[:, C_I:C_I + 128]
            H4X32 = consts[:, C_H4X32:C_H4X32 + 128]
            H4 = consts[:, C_H4:C_H4 + 4]
            ONES8 = consts[:, C_ONES8:C_ONES8 + 8]
            H4P8 = consts[:, C_H4P8:C_H4P8 + 8]
            ONES = consts[:, C_ONES:C_ONES + 1]
            masks = [consts[:, C_MASK + 512 * i: C_MASK + 512 * (i + 1)]
                     for i in range(4)]
            ctab = tabs[:, 0:T]
            stab = tabs[:, T:2 * T]

            bounce_in = dram.tile([NC, 4, 65, 256], F32)
            bounce_out = dram.tile([NC, 4, 65, 256], F32)

            # ============ PHASE 1+2: QKV + attention (per tq-chunk) ========
            with tc.tile_pool(name="p12", bufs=1) as p12, \
                 tc.tile_pool(name="ps12", bufs=1, space="PSUM") as ps12:
                wq = p12.tile([128, NKT, 256], F32R)
                nc.sync.dma_start(out=wq[:], in_=r3(wq_in))
                wk = p12.tile([128, NKT, 256], F32R)
                nc.sync.dma_start(out=wk[:], in_=r3(wk_in))
                wv = p12.tile([128, NKT, 256], F32R)
                nc.sync.dma_start(out=wv[:], in_=r3(wv_in))

                qpe1 = p12.tile([128, T], F32R)
                qpe2 = p12.tile([128, T], F32R)
                kpe1 = p12.tile([128, T], F32R)
                kpe2 = p12.tile([128, T], F32R)
                v4 = p12.tile([128, 16, 4, 65], F32R)
                nc.gpsimd.memset(v4[:, :, :, 64:65].bitcast(F32), 1.0)
                rk_c = p12.tile([128, 16, 4], F32)
                rv_c = p12.tile([128, 16], F32)

                for c in range(4):
                    ts = slice(512 * c, 512 * c + 512)
                    xc = p12.tile([128, NKT, 512], F32R, tag="xc", bufs=2,
                                  name=f"xc{c}")
                    nc.sync.dma_start(out=xc[:], in_=r3(xT_in)[:, :, ts])
                    # squares of x (for rv)
                    xsq = p12.tile([128, NKT, 512], F32R, tag="xsq", bufs=1,
                                   name=f"xsq{c}")
                    for k in range(NKT):
                        nc.gpsimd.tensor_mul(xsq[:, k], xc[:, k].bitcast(F32),
                             xc[:, k].bitcast(F32))
                    prv = ps12.tile([128, 32], F32, tag="psx", bufs=3,
                                    name=f"prv{c}")
                    for tt in range(4):
                        for k in range(NKT):
                            nc.tensor.matmul(
                                prv[:, 8 * tt:8 * tt + 8],
                                xsq[:, k, 128 * tt:128 * tt + 128], ONES8,
                                start=(k == 0), stop=(k == NKT - 1))
                    nc.scalar.activation(
                        rv_c[:, 4 * c:4 * c + 4],
                        prv[:].rearrange("p (t e) -> p t e", e=8)[:, :, 0],
                        AF.Abs_reciprocal_sqrt,
                        scale=1.0 / D, bias=eps[:, E_1EM6:E_1EM6 + 1])

                    # q projection (2 m-tiles) + per-head rsqrt + rope
                    pq = [ps12.tile([128, 512], F32, tag="pqk", bufs=2,
                                    name=f"pq{c}_{m}") for m in range(2)]
                    for m in range(2):
                        for k in range(NKT):
                            nc.tensor.matmul(
                                pq[m][:], wq[:, k, 128 * m:128 * m + 128],
                                xc[:, k], start=(k == 0), stop=(k == NKT - 1))
                    qsq = [p12.tile([128, 512], F32R, tag="qsq", bufs=4,
                                    name=f"qsq{c}_{m}") for m in range(2)]
                    for m in range(2):
                        nc.scalar.square(qsq[m][:], pq[m][:])
                    pssq = ps12.tile([128, 512], F32, tag="psx", bufs=3,
                                     name=f"pssq{c}")
                    nc.tensor.matmul(pssq[:], H4X32, qsq[0][:],
                                     start=True, stop=False)
                    nc.tensor.matmul(pssq[:], H4X32, qsq[1][:],
                                     start=False, stop=True)
                    bq = p12.tile([128, 512], F32, tag="bq", bufs=2,
                                  name=f"bq{c}")
                    nc.scalar.activation(
                        bq[:], pssq[:], AF.Abs_reciprocal_sqrt,
                        scale=1.0 / HD, bias=eps[:, E_1EM6:E_1EM6 + 1])
                    ra = p12.tile([128, 512], F32, tag="rt", bufs=3,
                                  name=f"ra{c}")
                    rb = p12.tile([128, 512], F32, tag="rt", bufs=3,
                                  name=f"rb{c}")
                    ro = p12.tile([128, 512], F32, tag="rt", bufs=3,
                                  name=f"ro{c}")
                    nc.vector.tensor_mul(ra[:], pq[0][:], ctab[:, ts])
                    nc.vector.tensor_mul(rb[:], pq[1][:], stab[:, ts])
                    nc.vector.tensor_sub(ro[:], ra[:], rb[:])
                    nc.vector.tensor_mul(qpe1[:, ts], ro[:], bq[:])
                    rc = p12.tile([128, 512], F32, tag="rt", bufs=3,
                                  name=f"rc{c}")
                    rd = p12.tile([128, 512], F32, tag="rt", bufs=3,
                                  name=f"rd{c}")
                    ro2 = p12.tile([128, 512], F32, tag="rt", bufs=3,
                                   name=f"ro2{c}")
                    nc.vector.tensor_mul(rc[:], pq[1][:], ctab[:, ts])
                    nc.vector.tensor_mul(rd[:], pq[0][:], stab[:, ts])
                    nc.vector.tensor_add(ro2[:], rc[:], rd[:])
                    nc.vector.tensor_mul(qpe2[:, ts], ro2[:], bq[:])

                    # k projection + rk cols + rope (unnormalized)
                    pk = [ps12.tile([128, 512], F32, tag="pqk", bufs=2,
                                    name=f"pk{c}_{m}") for m in range(2)]
                    for m in range(2):
                        for k in range(NKT):
                            nc.tensor.matmul(
                                pk[m][:], wk[:, k, 128 * m:128 * m + 128],
                                xc[:, k], start=(k == 0), stop=(k == NKT - 1))
                    ksq = [p12.tile([128, 512], F32R, tag="qsq", bufs=4,
                                    name=f"ksq{c}_{m}") for m in range(2)]
                    for m in range(2):
                        nc.scalar.square(ksq[m][:], pk[m][:])
                    prk = ps12.tile([128, 32], F32, tag="psx", bufs=3,
                                    name=f"prk{c}")
                    for tt in range(4):
                        nc.tensor.matmul(
                            prk[:, 8 * tt:8 * tt + 8],
                            ksq[0][:, 128 * tt:128 * tt + 128], H4P8,
                            start=True, stop=False)
                        nc.tensor.matmul(
                            prk[:, 8 * tt:8 * tt + 8],
                            ksq[1][:, 128 * tt:128 * tt + 128], H4P8,
                            start=False, stop=True)
                    nc.scalar.activation(
                        rk_c[:, 4 * c:4 * c + 4, :],
                        prk[:].rearrange("p (t h) -> p t h", h=8)[:, :, 0:4],
                        AF.Abs_reciprocal_sqrt,
                        scale=1.0, bias=eps[:, E_RK:E_RK + 1])
                    ka = p12.tile([128, 512], F32, tag="rt", bufs=3,
                                  name=f"ka{c}")
                    kb = p12.tile([128, 512], F32, tag="rt", bufs=3,
                                  name=f"kb{c}")
                    nc.vector.tensor_mul(ka[:], pk[0][:], ctab[:, ts])
                    nc.vector.tensor_mul(kb[:], pk[1][:], stab[:, ts])
                    nc.vector.tensor_sub(kpe1[:, ts], ka[:], kb[:])
                    kc = p12.tile([128, 512], F32, tag="rt", bufs=3,
                                  name=f"kc{c}")
                    kd = p12.tile([128, 512], F32, tag="rt", bufs=3,
                                  name=f"kd{c}")
                    nc.vector.tensor_mul(kc[:], pk[1][:], ctab[:, ts])
                    nc.vector.tensor_mul(kd[:], pk[0][:], stab[:, ts])
                    nc.vector.tensor_add(kpe2[:, ts], kc[:], kd[:])

                    # v projection, scaled by rv on psum->sbuf copy
                    for tt in range(4):
                        pv = ps12.tile([128, 256], F32, tag="pqk", bufs=2,
                                       name=f"pv{c}_{tt}")
                        for k in range(NKT):
                            nc.tensor.matmul(
                                pv[:], xc[:, k, 128 * tt:128 * tt + 128],
                                wv[:, k], start=(k == 0), stop=(k == NKT - 1))
                        nc.scalar.activation(
                            v4[:, 4 * c + tt, :, 0:64],
                            pv[:].rearrange("p (h d) -> p h d", d=64),
                            AF.Copy, scale=rv_c[:, 4 * c + tt:4 * c + tt + 1])

                    # ---- attention for tq-chunk c, head pairs {0,1}, {2,3}
                    njt = 4 * (c + 1)
                    for hp in range(2):
                        pO = [ps12.tile([65, 512], F32, tag="pO", bufs=2,
                                        name=f"pO{c}_{hp}_{hh}")
                              for hh in range(2)]
                        for j in range(njt):
                            ks = slice(128 * j, 128 * j + 128)
                            diag = j >= 4 * c
                            for hh in range(2):
                                h = 2 * hp + hh
                                hs = slice(32 * h, 32 * h + 32)
                                pS = ps12.tile([128, 512], F32, tag="psx",
                                               bufs=3, name=f"pS{c}_{hp}_{j}_{hh}")
                                if diag:
                                    nc.tensor.matmul(pS[:], I128,
                                                     masks[j - 4 * c],
                                                     start=True, stop=False)
                                nc.tensor.matmul(
                                    pS[:], kpe1[hs, ks], qpe1[hs, ts],
                                    start=not diag, stop=False,
                                    tile_position=(32 * h, 0))
                                nc.tensor.matmul(
                                    pS[:], kpe2[hs, ks], qpe2[hs, ts],
                                    start=False, stop=True,
                                    tile_position=(32 * h, 0))
                                pT = p12.tile([128, 512], F32R, tag="pT",
                                              bufs=3, name=f"pT{c}_{hp}_{j}_{hh}")
                                nc.scalar.activation(
                                    pT[:], pS[:], AF.Exp,
                                    scale=rk_c[:, j, h:h + 1])
                                nc.tensor.matmul(
                                    pO[hh][:], v4[:, j, h, :], pT[:],
                                    start=(j == 0), stop=(j == njt - 1))
                        for hh in range(2):
                            st = p12.tile([65, 512], F32, tag="st", bufs=3,
                                          name=f"st{c}_{hp}_{hh}")
                            nc.scalar.copy(st[:], pO[hh][:])
                            for half in range(2):
                                nc.sync.dma_start(
                                    out=bounce_in[2 * c + half, 2 * hp + hh],
                                    in_=st[:, 256 * half:256 * half + 256])

            # ================= AllToAll ====================================
            nc.gpsimd.collective_compute(
                "AllToAll", mybir.AluOpType.bypass,
                replica_groups=[list(range(NC))],
                ins=[bounce_in[:]], outs=[bounce_out[:]],
            )

            # ============ PHASE 3: O-proj + delta1 + MLP + delta2 ==========
            with tc.tile_pool(name="p3c", bufs=1) as p3c, \
                 tc.tile_pool(name="ps3", bufs=1, space="PSUM") as ps3:
                xown = p3c.tile([128, NKT, 512], F32R)
                nc.sync.dma_start(out=xown[:], in_=r3(xo_in))
                x1 = p3c.tile([128, NKT, 512], F32R)
                xm = p3c.tile([128, NKT, 512], F32R, tag="xmx2")
                hat = p3c.tile([128, NKT, 512], F32, tag="hbuf")
                gt = p3c.tile([128, NFT, 512], BF16)
                r2 = p3c.tile([1, 512], F32)

                def rows_tile(nm):
                    return p3c.tile([1, 512], F32, tag="rows", bufs=6, name=nm)

                def delta(xt, ht, out_t, cgw, cwv, e_gb, r_row, tagp):
                    phh = ps3.tile([1, 512], F32, tag="prow", bufs=3,
                                   name=f"phh{tagp}")
                    phx = ps3.tile([1, 512], F32, tag="prow", bufs=3,
                                   name=f"phx{tagp}")
                    for k in range(NKT):
                        hsq = p3c.tile([128, 512], F32R, tag="dt", bufs=4,
                                       name=f"hsq{tagp}{k}")
                        nc.gpsimd.tensor_mul(hsq[:], ht[:, k], ht[:, k])
                        nc.tensor.matmul(phh[:], ONES, hsq[:],
                                         start=(k == 0), stop=(k == NKT - 1))
                    for k in range(NKT):
                        hx = p3c.tile([128, 512], F32R, tag="dt", bufs=4,
                                      name=f"hx{tagp}{k}")
                        nc.gpsimd.tensor_mul(hx[:], ht[:, k],
                                             xt[:, k].bitcast(F32))
                        nc.tensor.matmul(phx[:], ONES, hx[:],
                                         start=(k == 0), stop=(k == NKT - 1))
                    pgx = ps3.tile([1, 512], F32, tag="prow", bufs=3,
                                   name=f"pgx{tagp}")
                    for k in range(NKT):
                        nc.tensor.matmul(pgx[:],
                                         consts[:, cgw + k:cgw + k + 1],
                                         xt[:, k],
                                         start=(k == 0), stop=(k == NKT - 1))
                    pwx = ps3.tile([1, 512], F32, tag="prow", bufs=3,
                                   name=f"pwx{tagp}")
                    for k in range(NKT):
                        nc.tensor.matmul(pwx[:],
                                         consts[:, cwv + k:cwv + k + 1],
                                         xt[:, k],
                                         start=(k == 0), stop=(k == NKT - 1))
                    cs = rows_tile(f"cs{tagp}")
                    nc.scalar.activation(cs[:], phh[:], AF.Abs_reciprocal_sqrt,
                                         scale=1024.0,
                                         bias=eps[0:1, E_CS:E_CS + 1])
                    vg = rows_tile(f"vg{tagp}")
                    nc.scalar.activation(vg[:], pwx[:], AF.Sigmoid)
                    lg = rows_tile(f"lg{tagp}")
                    nc.vector.tensor_mul(lg[:], pgx[:], r_row[:])
                    sg = rows_tile(f"sg{tagp}")
                    nc.scalar.activation(sg[:], lg[:], AF.Sigmoid,
                                         bias=eps[0:1, e_gb:e_gb + 1])
                    kx = rows_tile(f"kx{tagp}")
                    nc.vector.tensor_mul(kx[:], phx[:], cs[:])
                    t2 = rows_tile(f"t2{tagp}")
                    nc.vector.tensor_sub(t2[:], vg[:], kx[:])
                    t3 = rows_tile(f"t3{tagp}")
                    nc.vector.tensor_mul(t3[:], t2[:], sg[:])
                    t4 = rows_tile(f"t4{tagp}")
                    nc.vector.tensor_mul(t4[:], t3[:], cs[:])
                    coef = rows_tile(f"coef{tagp}")
                    nc.scalar.mul(coef[:], t4[:], 2.0)
                    bco = p3c.tile([128, 512], F32, tag="bco", bufs=2,
                                   name=f"bco{tagp}")
                    nc.gpsimd.partition_broadcast(bco[:], coef[:])
                    for k in range(NKT):
                        tm = p3c.tile([128, 512], F32, tag="dt", bufs=4,
                                      name=f"tm{tagp}{k}")
                        nc.vector.tensor_mul(tm[:], bco[:], ht[:, k])
                        nc.vector.tensor_add(out_t[:, k], tm[:],
                                             xt[:, k].bitcast(F32))

                with tc.tile_pool(name="p3a", bufs=1) as p3a:
                    sall = p3a.tile([16, 512], F32)
                    for i in range(NC):
                        bi, m = i // 4, i % 4
                        nc.sync.dma_start(
                            out=sall[4 * m:4 * m + 4,
                                     256 * bi:256 * bi + 256],
                            in_=bounce_out[i, :, 64, :])
                    s2 = p3a.tile([16, 512], F32)
                    nc.vector.tensor_mul(s2[:], sall[:], sall[:])
                    rT = p3a.tile([16, 512], F32R)
                    nc.scalar.activation(rT[:], s2[:], AF.Abs_reciprocal_sqrt,
                                         scale=1.0,
                                         bias=eps[0:16, E_RT:E_RT + 1])
                    aT = p3a.tile([128, NKT, 512], F32R)
                    for kt in range(NKT):
                        ar = p3a.tile([128, 512], F32, tag="ar", bufs=2,
                                      name=f"ar{kt}")
                        for bi in range(2):
                            for hh in range(2):
                                hg = 2 * kt + hh
                                i = bi * 4 + hg // 4
                                nc.sync.dma_start(
                                    out=ar[64 * hh:64 * hh + 64,
                                           256 * bi:256 * bi + 256],
                                    in_=bounce_out[i, hg % 4, 0:64, :])
                        pbc = ps3.tile([128, 512], F32, tag="pbig", bufs=4,
                                       name=f"pbc{kt}")
                        nc.tensor.matmul(
                            pbc[:],
                            consts[0:16, C_SEL + 128 * kt:C_SEL + 128 * (kt + 1)],
                            rT[:], start=True, stop=True)
                        nc.vector.tensor_mul(aT[:, kt], ar[:], pbc[:])

                    for m in range(NKT):
                        wot = p3a.tile([128, NKT, 128], F32R, tag="wot",
                                       bufs=3, name=f"wot{m}")
                        nc.sync.dma_start(
                            out=wot[:],
                            in_=r3(wo_in)[:, :, 128 * m:128 * m + 128])
                        po = ps3.tile([128, 512], F32, tag="pbig", bufs=4,
                                      name=f"po{m}")
                        for k in range(NKT):
                            nc.tensor.matmul(po[:], wot[:, k], aT[:, k],
                                             start=(k == 0),
                                             stop=(k == NKT - 1))
                        nc.scalar.copy(hat[:, m], po[:])

                    # r1 row for delta1 (over x_own)
                    pso = ps3.tile([1, 512], F32, tag="prow", bufs=3,
                                   name="pso")
                    for k in range(NKT):
                        xq = p3c.tile([128, 512], F32R, tag="dt", bufs=4,
                                      name=f"xosq{k}")
                        nc.gpsimd.tensor_mul(xq[:], xown[:, k].bitcast(F32),
                                             xown[:, k].bitcast(F32))
                        nc.tensor.matmul(pso[:], ONES, xq[:],
                                         start=(k == 0), stop=(k == NKT - 1))
                    r1 = p3a.tile([1, 512], F32, name="r1")
                    nc.scalar.activation(r1[:], pso[:], AF.Abs_reciprocal_sqrt,
                                         scale=1.0 / D,
                                         bias=eps[0:1, E_1EM6:E_1EM6 + 1])
                    delta(xown, hat, x1, C_GW1, C_WV1, E_G1B, r1, "d1")

                # r2 + xm
                ps1 = ps3.tile([1, 512], F32, tag="prow", bufs=3, name="ps1")
                for k in range(NKT):
                    xq = p3c.tile([128, 512], F32R, tag="dt", bufs=4,
                                  name=f"x1sq{k}")
                    nc.gpsimd.tensor_mul(xq[:], x1[:, k].bitcast(F32),
                                         x1[:, k].bitcast(F32))
                    nc.tensor.matmul(ps1[:], ONES, xq[:],
                                     start=(k == 0), stop=(k == NKT - 1))
                nc.scalar.activation(r2[:], ps1[:], AF.Abs_reciprocal_sqrt,
                                     scale=1.0 / D,
                                     bias=eps[0:1, E_1EM6:E_1EM6 + 1])
                br2 = p3c.tile([128, 512], F32, tag="bco", bufs=2, name="br2")
                nc.gpsimd.partition_broadcast(br2[:], r2[:])
                for k in range(NKT):
                    nc.vector.tensor_mul(xm[:, k], x1[:, k].bitcast(F32),
                                         br2[:])

                # MLP
                with tc.tile_pool(name="p3b", bufs=1) as p3b:
                    for m in range(NFT):
                        w1t = p3b.tile([128, NKT, 128], F32R, tag="w1t",
                                       bufs=2, name=f"w1t{m}")
                        nc.sync.dma_start(
                            out=w1t[:],
                            in_=r3(w1_in)[:, :, 128 * m:128 * m + 128])
                        ph1 = ps3.tile([128, 512], F32, tag="pbig", bufs=4,
                                       name=f"ph1_{m}")
                        for k in range(NKT):
                            nc.tensor.matmul(ph1[:], w1t[:, k], xm[:, k],
                                             start=(k == 0),
                                             stop=(k == NKT - 1))
                        gs = p3b.tile([128, 512], F32R, tag="gs", bufs=3,
                                      name=f"gs{m}")
                        nc.scalar.activation(gs[:], ph1[:], AF.Silu)
                        w3t = p3b.tile([128, NKT, 128], F32R, tag="w3t",
                                       bufs=2, name=f"w3t{m}")
                        nc.sync.dma_start(
                            out=w3t[:],
                            in_=r3(w3_in)[:, :, 128 * m:128 * m + 128])
                        ph3 = ps3.tile([128, 512], F32, tag="pbig", bufs=4,
                                       name=f"ph3_{m}")
                        for k in range(NKT):
                            nc.tensor.matmul(ph3[:], w3t[:, k], xm[:, k],
                                             start=(k == 0),
                                             stop=(k == NKT - 1))
                        nc.vector.tensor_mul(gt[:, m], gs[:].bitcast(F32),
                                             ph3[:])

                    hmlp = p3c.tile([128, NKT, 512], F32, tag="hbuf",
                                    name="hmlp")
                    for m2 in range(NKT):
                        w2t = p3b.tile([128, NFT, 128], BF16, tag="w2t",
                                       bufs=2, name=f"w2t{m2}")
                        nc.sync.dma_start(
                            out=w2t[:],
                            in_=w2_in[:].rearrange("(f p) t -> p f t", p=128)[
                                :, :, 128 * m2:128 * m2 + 128])
                        py = ps3.tile([128, 512], F32, tag="pbig", bufs=4,
                                      name=f"py{m2}")
                        for f in range(NFT):
                            nc.tensor.matmul(py[:], w2t[:, f], gt[:, f],
                                             start=(f == 0),
                                             stop=(f == NFT - 1))
                        nc.scalar.copy(hmlp[:, m2], py[:])

                    x2 = p3c.tile([128, NKT, 512], F32, tag="xmx2",
                                  name="x2")
                    delta(x1, hmlp, x2, C_GW2, C_WV2, E_G2B, r2, "d2")
                    nc.sync.dma_start(out=r3(out_d), in_=x2[:])

    nc.compile()
    return nc


def _host_prep(inputs):
    f = {k: np.ascontiguousarray(np.asarray(v, np.float32))
         for k, v in inputs.items()}
    anw, mnw = f["attn_norm_w"], f["mlp_norm_w"]
    qn, kn = f["qn_w"], f["kn_w"]
    assert np.allclose(qn[:32], qn[32:]) and np.allclose(qn, kn), \
        "kernel assumes qn_w/kn_w with equal halves (rope-foldable)"
    dd = np.arange(32)
    inv_freq = 1.0 / (10000.0 ** (np.arange(0, HD, 2) / HD))
    t = np.arange(T)
    cos = np.cos(t[None, :] * inv_freq[:, None]).astype(np.float32)
    sin = np.sin(t[None, :] * inv_freq[:, None]).astype(np.float32)
    ctab = np.tile(cos * qn[:32, None], (4, 1)).astype(np.float32)
    stab = np.tile(sin * qn[:32, None], (4, 1)).astype(np.float32)
    tabs = np.concatenate([ctab, stab], axis=1)

    # consts blob
    consts = np.zeros((128, CCOLS), np.float32)
    consts[:, C_I:C_I + 128] = np.eye(128)
    p = np.arange(128)
    h4x32 = (p[:, None] // 32 == np.arange(128)[None, :] // 32).astype(np.float32)
    consts[:, C_H4X32:C_H4X32 + 128] = h4x32
    consts[:, C_H4:C_H4 + 4] = (p[:, None] // 32 == np.arange(4)[None, :])
    consts[:, C_ONES] = 1.0
    consts[:, C_ONES8:C_ONES8 + 8] = 1.0
    consts[:, C_H4P8:C_H4P8 + 4] = (p[:, None] // 32 == np.arange(4)[None, :])
    i_ = np.arange(128)[:, None]
    jj = np.arange(512)[None, :]
    for dk in range(4):
        consts[:, C_MASK + 512 * dk:C_MASK + 512 * (dk + 1)] = np.where(
            jj >= i_ + 128 * dk, 0.0, MASKVAL)
    gwf1 = (f["g1_norm_w"] * f["g1_w"]).reshape(NKT, 128).T
    wv1 = f["wv1"].reshape(NKT, 128).T
    gwf2 = (f["g2_norm_w"] * f["g2_w"]).reshape(NKT, 128).T
    wv2 = f["wv2"].reshape(NKT, 128).T
    consts[:, C_GW1:C_GW1 + 8] = gwf1
    consts[:, C_WV1:C_WV1 + 8] = wv1
    consts[:, C_GW2:C_GW2 + 8] = gwf2
    consts[:, C_WV2:C_WV2 + 8] = wv2
    for kt in range(NKT):
        sel = np.zeros((128, 128), np.float32)
        mm = np.arange(128)
        sel[2 * kt, mm < 64] = 1.0
        sel[2 * kt + 1, mm >= 64] = 1.0
        consts[:, C_SEL + 128 * kt:C_SEL + 128 * (kt + 1)] = sel

    eps = np.zeros((128, 8), np.float32)
    eps[:, E_1EM6] = 1e-6
    eps[:, E_RK] = HD * 1e-6
    eps[:, E_CS] = 1e-9
    eps[:, E_RT] = 1e-30
    eps[:, E_G1B] = float(np.ravel(f["g1_b"])[0])
    eps[:, E_G2B] = float(np.ravel(f["g2_b"])[0])
    eps[:, E_TWO] = 2.0

    wq_f = f["wq"] * anw[None, :]
    wk_f = f["wk"] * anw[None, :]
    wv_f = f["wv"] * anw[None, :]
    w1p = np.zeros((FHP, D), np.float32)
    w1p[:FH] = f["w1"] * mnw[None, :]
    w3p = np.zeros((FHP, D), np.float32)
    w3p[:FH] = f["w3"] * mnw[None, :]
    w2p = np.zeros((FHP, D), np.float32)
    w2p[:FH] = f["w2"].T          # w2_sb = w2.T padded: [FHP, D]
    w1_sb = np.ascontiguousarray(w1p.T)   # [D, FHP]
    w3_sb = np.ascontiguousarray(w3p.T)
    w2_sb = np.ascontiguousarray(w2p).astype(ml_dtypes.bfloat16)
    wo_sb = np.ascontiguousarray(f["wo"].T)

    in_maps = []
    for g in range(NC):
        b, j = g // 4, g % 4
        heads = np.arange(4 * j, 4 * j + 4)
        rows_x1 = (heads[:, None] * HD + dd[None, :]).ravel()
        rows_x2 = (heads[:, None] * HD + 32 + dd[None, :]).ravel()
        rows_split = np.concatenate([rows_x1, rows_x2])
        rows_nat = (heads[:, None] * HD + np.arange(HD)[None, :]).ravel()
        x_own = np.concatenate(
            [f["x"][0].T[:, 256 * g:256 * g + 256],
             f["x"][1].T[:, 256 * g:256 * g + 256]], axis=1)
        in_maps.append({
            "xT": np.ascontiguousarray(f["x"][b].T),
            "x_own": np.ascontiguousarray(x_own),
            "wq_sb": np.ascontiguousarray(wq_f[rows_split].T),
            "wk_sb": np.ascontiguousarray(wk_f[rows_split].T),
            "wv_sb": np.ascontiguousarray(wv_f[rows_nat].T),
            "wo_sb": wo_sb,
            "w1_sb": w1_sb,
            "w3_sb": w3_sb,
            "w2_sb": w2_sb,
            "consts": consts,
            "tabs": tabs,
            "eps": eps,
        })
    return in_maps


def kernel(**inputs):
    if "nc" not in _CACHE:
        _CACHE["nc"] = _build_nc()
    nc = _CACHE["nc"]
    in_maps = _host_prep(inputs)
    res = run_bass_kernel_spmd(nc, in_maps, list(range(NC)))
    _CACHE["last_results"] = res
    out = np.zeros((B, T, D), np.float32)
    for g in range(NC):
        xo = res.results[g]["x_out"]          # [D, 512]
        out[0, 256 * g:256 * g + 256, :] = xo[:, 0:256].T
        out[1, 256 * g:256 * g + 256, :] = xo[:, 256:512].T
    return out


# revision 30
# speedup vs baseline: 17384.8415x; 17384.8415x over previous
"""Trainium2 Bass kernel for nn_DDLTransformerBlock (8 NeuronCores).

Sharding: core g in 0..7 -> batch b=g//4, heads [4*(g%4), 4*(g%4)+4) for the
attention part (feature-major, fp32r matmuls, split-x1/x2 rope layout, rk
folded into the exp scale); one 8-core AllToAll redistributes attention
output head-blocks to token owners (each core owns a 256-token slice of
EACH batch); O-projection, delta-residual, SwiGLU MLP and the second delta
run token-sharded with fully replicated weights.
"""
import ml_dtypes
import numpy as np
import concourse.bacc as bacc
import concourse.mybir as mybir
from concourse.tile import TileContext
from concourse.bass_utils import run_bass_kernel_spmd

B, T, D, H, HD, FH = 2, 2048, 1024, 16, 64, 2752
FHP = 2816           # FH padded to 22*128
NKT = D // 128       # 8 k-tiles
NFT = FHP // 128     # 22
NC = 8
MASKVAL = -1e6
F32 = mybir.dt.float32
F32R = mybir.dt.float32r
BF16 = mybir.dt.bfloat16
AF = mybir.ActivationFunctionType

# consts blob column layout (fp32r, [128, CCOLS])
C_I = 0            # identity 128
C_H4X32 = 128      # 128
C_H4 = 256         # 4
C_ONES = 260       # 1
C_MASK = 261       # 4*512
C_GW1 = C_MASK + 4 * 512   # 8
C_WV1 = C_GW1 + 8          # 8
C_GW2 = C_WV1 + 8          # 8
C_WV2 = C_GW2 + 8          # 8
C_SEL = C_WV2 + 8          # 8*128 (sel_kt: [16,128] blocks)
C_ONES8 = C_SEL + 8 * 128  # 8 (all-ones cols)
C_H4P8 = C_ONES8 + 8       # 8 (H4 padded to 8 cols)
CCOLS = C_H4P8 + 8
# eps tile columns (f32, [128, 8])
E_1EM6, E_RK, E_CS, E_RT, E_G1B, E_G2B, E_ZERO, E_TWO = range(8)

_CACHE = {}


def _build_nc():
    nc = bacc.Bacc("TRN2", target_bir_lowering=False, num_devices=NC)
    xT_in = nc.declare_dram_parameter("xT", [D, T], F32R, isOutput=False)
    xo_in = nc.declare_dram_parameter("x_own", [D, 512], F32R, isOutput=False)
    wq_in = nc.declare_dram_parameter("wq_sb", [D, 256], F32R, isOutput=False)
    wk_in = nc.declare_dram_parameter("wk_sb", [D, 256], F32R, isOutput=False)
    wv_in = nc.declare_dram_parameter("wv_sb", [D, 256], F32R, isOutput=False)
    wo_in = nc.declare_dram_parameter("wo_sb", [D, D], F32R, isOutput=False)
    w1_in = nc.declare_dram_parameter("w1_sb", [D, FHP], F32R, isOutput=False)
    w3_in = nc.declare_dram_parameter("w3_sb", [D, FHP], F32R, isOutput=False)
    w2_in = nc.declare_dram_parameter("w2_sb", [FHP, D], BF16, isOutput=False)
    cn_in = nc.declare_dram_parameter("consts", [128, CCOLS], F32R, isOutput=False)
    tb_in = nc.declare_dram_parameter("tabs", [128, 2 * T], F32, isOutput=False)
    ep_in = nc.declare_dram_parameter("eps", [128, 8], F32, isOutput=False)
    out_d = nc.declare_dram_parameter("x_out", [D, 512], F32, isOutput=True)

    def r3(dram, p=128):
        # view [R, C] dram as (p, ktile, C) for tile DMAs
        return dram[:].rearrange("(k p) t -> p k t", p=p)

    with TileContext(nc) as tc:
        with tc.tile_pool(name="glob", bufs=1) as glob, \
             tc.tile_pool(name="dram", bufs=1, space="DRAM") as dram:
            consts = glob.tile([128, CCOLS], F32R)
            nc.sync.dma_start(out=consts[:], in_=cn_in[:])
            eps = glob.tile([128, 8], F32)
            nc.sync.dma_start(out=eps[:], in_=ep_in[:])
            xown = glob.tile([128, NKT, 512], F32R)
            nc.sync.dma_start(out=xown[:], in_=r3(xo_in))

            I128 = consts[:, C_I:C_I + 128]
            H4X32 = consts[:, C_H4X32:C_H4X32 + 128]
            H4 = consts[:, C_H4:C_H4 + 4]
            ONES8 = consts[:, C_ONES8:C_ONES8 + 8]
            H4P8 = consts[:, C_H4P8:C_H4P8 + 8]
            ONES = consts[:, C_ONES:C_ONES + 1]
            masks = [consts[:, C_MASK + 512 * i: C_MASK + 512 * (i + 1)]
                     for i in range(4)]

            SHW = 128 * 2 * 64 + 4 * 64   # shard words: features + sums
            bounce_in = [dram.tile([NC, SHW], F32, name=f"bin{c}")
                         for c in range(4)]
            bounce_out = [dram.tile([NC, SHW], F32, name=f"bout{c}")
                          for c in range(4)]
            araw = glob.tile([128, NKT, 512], F32)
            sall = glob.tile([16, 512], F32)

            # ============ PHASE 1+2: QKV + attention (per tq-chunk) ========
            with tc.tile_pool(name="p12", bufs=1) as p12, \
                 tc.tile_pool(name="ps12", bufs=1, space="PSUM") as ps12:
                tabs = p12.tile([128, 2 * T], F32)
                nc.sync.dma_start(out=tabs[:], in_=tb_in[:])
                ctab = tabs[:, 0:T]
                stab = tabs[:, T:2 * T]
                wq = p12.tile([128, NKT, 256], F32R)
                nc.sync.dma_start(out=wq[:], in_=r3(wq_in))
                wk = p12.tile([128, NKT, 256], F32R)
                nc.sync.dma_start(out=wk[:], in_=r3(wk_in))
                wv = p12.tile([128, NKT, 256], F32R)
                nc.sync.dma_start(out=wv[:], in_=r3(wv_in))

                qpe1 = p12.tile([128, T], F32R)
                qpe2 = p12.tile([128, T], F32R)
                kpe1 = p12.tile([128, T], F32R)
                kpe2 = p12.tile([128, T], F32R)
                v4 = p12.tile([128, 16, 4, 65], F32R)
                nc.gpsimd.memset(v4[:, :, :, 64:65].bitcast(F32), 1.0)
                rk_c = p12.tile([128, 16, 4], F32)
                rv_c = p12.tile([128, 16], F32)

                for c in range(4):
                    ts = slice(512 * c, 512 * c + 512)
                    xc = p12.tile([128, NKT, 512], F32R, tag="xc", bufs=2,
                                  name=f"xc{c}")
                    nc.sync.dma_start(out=xc[:], in_=r3(xT_in)[:, :, ts])
                    # squares of x (for rv), rotating per k-tile
                    prv = ps12.tile([128, 32], F32, tag="psx", bufs=3,
                                    name=f"prv{c}")
                    for k in range(NKT):
                        xsq = p12.tile([128, 512], F32R, tag="xsq", bufs=2,
                                       name=f"xsq{c}_{k}")
                        nc.gpsimd.tensor_mul(xsq[:], xc[:, k].bitcast(F32),
                                             xc[:, k].bitcast(F32))
                        for tt in range(4):
                            nc.tensor.matmul(
                                prv[:, 8 * tt:8 * tt + 8],
                                xsq[:, 128 * tt:128 * tt + 128], ONES8,
                                start=(k == 0), stop=(k == NKT - 1))
                    nc.scalar.activation(
                        rv_c[:, 4 * c:4 * c + 4],
                        prv[:].rearrange("p (t e) -> p t e", e=8)[:, :, 0],
                        AF.Abs_reciprocal_sqrt,
                        scale=1.0 / D, bias=eps[:, E_1EM6:E_1EM6 + 1])

                    # q projection (2 m-tiles) + per-head rsqrt + rope
                    pq = [ps12.tile([128, 512], F32, tag="pqk", bufs=2,
                                    name=f"pq{c}_{m}") for m in range(2)]
                    for m in range(2):
                        for k in range(NKT):
                            nc.tensor.matmul(
                                pq[m][:], wq[:, k, 128 * m:128 * m + 128],
                                xc[:, k], start=(k == 0), stop=(k == NKT - 1))
                    qsq = [p12.tile([128, 512], F32R, tag="qsq", bufs=4,
                                    name=f"qsq{c}_{m}") for m in range(2)]
                    for m in range(2):
                        nc.scalar.square(qsq[m][:], pq[m][:])
                    pssq = ps12.tile([128, 512], F32, tag="psx", bufs=3,
                                     name=f"pssq{c}")
                    nc.tensor.matmul(pssq[:], H4X32, qsq[0][:],
                                     start=True, stop=False)
                    nc.tensor.matmul(pssq[:], H4X32, qsq[1][:],
                                     start=False, stop=True)
                    bq = p12.tile([128, 512], F32, tag="bq", bufs=2,
                                  name=f"bq{c}")
                    nc.scalar.activation(
                        bq[:], pssq[:], AF.Abs_reciprocal_sqrt,
                        scale=1.0 / HD, bias=eps[:, E_1EM6:E_1EM6 + 1])
                    ra = p12.tile([128, 512], F32, tag="rt", bufs=3,
                                  name=f"ra{c}")
                    rb = p12.tile([128, 512], F32, tag="rt", bufs=3,
                                  name=f"rb{c}")
                    ro = p12.tile([128, 512], F32, tag="rt", bufs=3,
                                  name=f"ro{c}")
                    nc.vector.tensor_mul(ra[:], pq[0][:], ctab[:, ts])
                    nc.vector.tensor_mul(rb[:], pq[1][:], stab[:, ts])
                    nc.vector.tensor_sub(ro[:], ra[:], rb[:])
                    nc.vector.tensor_mul(qpe1[:, ts], ro[:], bq[:])
                    rc = p12.tile([128, 512], F32, tag="rt", bufs=3,
                                  name=f"rc{c}")
                    rd = p12.tile([128, 512], F32, tag="rt", bufs=3,
                                  name=f"rd{c}")
                    ro2 = p12.tile([128, 512], F32, tag="rt", bufs=3,
                                   name=f"ro2{c}")
                    nc.vector.tensor_mul(rc[:], pq[1][:], ctab[:, ts])
                    nc.vector.tensor_mul(rd[:], pq[0][:], stab[:, ts])
                    nc.vector.tensor_add(ro2[:], rc[:], rd[:])
                    nc.vector.tensor_mul(qpe2[:, ts], ro2[:], bq[:])

                    # k projection + rk cols + rope (unnormalized)
                    pk = [ps12.tile([128, 512], F32, tag="pqk", bufs=2,
                                    name=f"pk{c}_{m}") for m in range(2)]
                    for m in range(2):
                        for k in range(NKT):
                            nc.tensor.matmul(
                                pk[m][:], wk[:, k, 128 * m:128 * m + 128],
                                xc[:, k], start=(k == 0), stop=(k == NKT - 1))
                    ksq = [p12.tile([128, 512], F32R, tag="qsq", bufs=4,
                                    name=f"ksq{c}_{m}") for m in range(2)]
                    for m in range(2):
                        nc.scalar.square(ksq[m][:], pk[m][:])
                    prk = ps12.tile([128, 32], F32, tag="psx", bufs=3,
                                    name=f"prk{c}")
                    for tt in range(4):
                        nc.tensor.matmul(
                            prk[:, 8 * tt:8 * tt + 8],
                            ksq[0][:, 128 * tt:128 * tt + 128], H4P8,
                            start=True, stop=False)
                        nc.tensor.matmul(
                            prk[:, 8 * tt:8 * tt + 8],
                            ksq[1][:, 128 * tt:128 * tt + 128], H4P8,
                            start=False, stop=True)
                    nc.scalar.activation(
                        rk_c[:, 4 * c:4 * c + 4, :],
                        prk[:].rearrange("p (t h) -> p t h", h=8)[:, :, 0:4],
                        AF.Abs_reciprocal_sqrt,
                        scale=1.0, bias=eps[:, E_RK:E_RK + 1])
                    ka = p12.tile([128, 512], F32, tag="rt", bufs=3,
                                  name=f"ka{c}")
                    kb = p12.tile([128, 512], F32, tag="rt", bufs=3,
                                  name=f"kb{c}")
                    nc.vector.tensor_mul(ka[:], pk[0][:], ctab[:, ts])
                    nc.vector.tensor_mul(kb[:], pk[1][:], stab[:, ts])
                    nc.vector.tensor_sub(kpe1[:, ts], ka[:], kb[:])
                    kc = p12.tile([128, 512], F32, tag="rt", bufs=3,
                                  name=f"kc{c}")
                    kd = p12.tile([128, 512], F32, tag="rt", bufs=3,
                                  name=f"kd{c}")
                    nc.vector.tensor_mul(kc[:], pk[1][:], ctab[:, ts])
                    nc.vector.tensor_mul(kd[:], pk[0][:], stab[:, ts])
                    nc.vector.tensor_add(kpe2[:, ts], kc[:], kd[:])

                    # v projection, scaled by rv on psum->sbuf copy
                    for tt in range(4):
                        pv = ps12.tile([128, 256], F32, tag="pqk", bufs=2,
                                       name=f"pv{c}_{tt}")
                        for k in range(NKT):
                            nc.tensor.matmul(
                                pv[:], xc[:, k, 128 * tt:128 * tt + 128],
                                wv[:, k], start=(k == 0), stop=(k == NKT - 1))
                        nc.vector.tensor_scalar_mul(
                            v4[:, 4 * c + tt, :, 0:64],
                            pv[:].rearrange("p (h d) -> p h d", d=64),
                            rv_c[:, 4 * c + tt:4 * c + tt + 1])

                    # ---- attention for tq-chunk c, head pairs {0,1}, {2,3}
                    njt = 4 * (c + 1)
                    for hp in range(2):
                        pO = [ps12.tile([65, 512], F32, tag="pO", bufs=2,
                                        name=f"pO{c}_{hp}_{hh}")
                              for hh in range(2)]
                        for j in range(njt):
                            ks = slice(128 * j, 128 * j + 128)
                            diag = j >= 4 * c
                            for hh in range(2):
                                h = 2 * hp + hh
                                hs = slice(32 * h, 32 * h + 32)
                                pS = ps12.tile([128, 512], F32, tag="psx",
                                               bufs=3, name=f"pS{c}_{hp}_{j}_{hh}")
                                if diag:
                                    nc.tensor.matmul(pS[:], I128,
                                                     masks[j - 4 * c],
                                                     start=True, stop=False)
                                nc.tensor.matmul(
                                    pS[:], kpe1[hs, ks], qpe1[hs, ts],
                                    start=not diag, stop=False,
                                    tile_position=(32 * h, 0))
                                nc.tensor.matmul(
                                    pS[:], kpe2[hs, ks], qpe2[hs, ts],
                                    start=False, stop=True,
                                    tile_position=(32 * h, 0))
                                pT = p12.tile([128, 512], F32R, tag="pT",
                                              bufs=3, name=f"pT{c}_{hp}_{j}_{hh}")
                                nc.scalar.activation(
                                    pT[:], pS[:], AF.Exp,
                                    scale=rk_c[:, j, h:h + 1])
                                nc.tensor.matmul(
                                    pO[hh][:], v4[:, j, h, :], pT[:],
                                    start=(j == 0), stop=(j == njt - 1))
                        for hh in range(2):
                            h = 2 * hp + hh
                            st = p12.tile([65, 512], F32, tag="st", bufs=3,
                                          name=f"st{c}_{hp}_{hh}")
                            nc.vector.tensor_copy(st[:], pO[hh][:])
                            qv = bounce_in[c][:].rearrange(
                                "s (q t) -> s q t", t=64)
                            fview = qv[:, 0:256, :].rearrange(
                                "s (p k) t -> s p k t", k=2)
                            nc.sync.dma_start(
                                out=fview[:, 64 * (h % 2):64 * (h % 2) + 64,
                                          h // 2, :].transpose([1, 0, 2]),
                                in_=st[0:64, :].rearrange(
                                    "p (s t) -> p s t", t=64))
                            nc.sync.dma_start(
                                out=qv[:, 256 + h, :].unsqueeze(0),
                                in_=st[64:65, :].rearrange(
                                    "p (s t) -> p s t", t=64))
                    nc.gpsimd.collective_compute(
                        "AllToAll", mybir.AluOpType.bypass,
                        replica_groups=[list(range(NC))],
                        ins=[bounce_in[c][:]], outs=[bounce_out[c][:]],
                    )
                    # receive chunk-c shards (overlaps later chunks)
                    for i in range(NC):
                        bi, m = i // 4, i % 4
                        col = 256 * bi + 64 * c
                        qview = bounce_out[c][:].rearrange(
                            "s (q t) -> s q t", t=64)
                        nc.sync.dma_start(
                            out=araw[:, 2 * m:2 * m + 2, col:col + 64],
                            in_=qview[i, 0:256, :].rearrange(
                                "(p k) t -> p k t", k=2))
                        nc.sync.dma_start(
                            out=sall[4 * m:4 * m + 4, col:col + 64],
                            in_=qview[i, 256:260, :])

            # ============ PHASE 3: O-proj + delta1 + MLP + delta2 ==========
            with tc.tile_pool(name="p3c", bufs=1) as p3c, \
                 tc.tile_pool(name="ps3", bufs=1, space="PSUM") as ps3:
                xown = p3c.tile([128, NKT, 512], F32R)
                nc.sync.dma_start(out=xown[:], in_=r3(xo_in))
                x1 = p3c.tile([128, NKT, 512], F32R)
                xm = p3c.tile([128, NKT, 512], F32R, tag="xmx2")
                hat = p3c.tile([128, NKT, 512], F32, tag="hbuf")
                gt = p3c.tile([128, NFT, 512], BF16)
                r2 = p3c.tile([1, 512], F32)

                def rows_tile(nm):
                    return p3c.tile([1, 512], F32, tag="rows", bufs=5, name=nm)

                # delta1 reductions that depend only on x_own: emit early so
                # PE fills the tail of the attention/A2A window
                pso = ps3.tile([1, 512], F32, tag="prow", bufs=3, name="pso")
                for k in range(NKT):
                    xq = p3c.tile([128, 512], F32R, tag="dt", bufs=3,
                                  name=f"xosq{k}")
                    nc.gpsimd.tensor_mul(xq[:], xown[:, k].bitcast(F32),
                                         xown[:, k].bitcast(F32))
                    nc.tensor.matmul(pso[:], ONES, xq[:],
                                     start=(k == 0), stop=(k == NKT - 1))
                r1 = p3c.tile([1, 512], F32, name="r1")
                nc.scalar.activation(r1[:], pso[:], AF.Abs_reciprocal_sqrt,
                                     scale=1.0 / D,
                                     bias=eps[0:1, E_1EM6:E_1EM6 + 1])
                pgx1 = ps3.tile([1, 512], F32, tag="prow", bufs=3,
                                name="pgx1")
                for k in range(NKT):
                    nc.tensor.matmul(pgx1[:],
                                     consts[:, C_GW1 + k:C_GW1 + k + 1],
                                     xown[:, k],
                                     start=(k == 0), stop=(k == NKT - 1))
                pwx1 = ps3.tile([1, 512], F32, tag="prow", bufs=3,
                                name="pwx1")
                for k in range(NKT):
                    nc.tensor.matmul(pwx1[:],
                                     consts[:, C_WV1 + k:C_WV1 + k + 1],
                                     xown[:, k],
                                     start=(k == 0), stop=(k == NKT - 1))

                def delta(xt, ht, out_t, cgw, cwv, e_gb, r_row, tagp,
                          pgx=None, pwx=None):
                    phh = ps3.tile([1, 512], F32, tag="prow", bufs=3,
                                   name=f"phh{tagp}")
                    phx = ps3.tile([1, 512], F32, tag="prow", bufs=3,
                                   name=f"phx{tagp}")
                    for k in range(NKT):
                        hsq = p3c.tile([128, 512], F32R, tag="dt", bufs=3,
                                       name=f"hsq{tagp}{k}")
                        nc.gpsimd.tensor_mul(hsq[:], ht[:, k], ht[:, k])
                        nc.tensor.matmul(phh[:], ONES, hsq[:],
                                         start=(k == 0), stop=(k == NKT - 1))
                    for k in range(NKT):
                        hx = p3c.tile([128, 512], F32R, tag="dt", bufs=3,
                                      name=f"hx{tagp}{k}")
                        nc.gpsimd.tensor_mul(hx[:], ht[:, k],
                                             xt[:, k].bitcast(F32))
                        nc.tensor.matmul(phx[:], ONES, hx[:],
                                         start=(k == 0), stop=(k == NKT - 1))
                    if pgx is None:
                        pgx = ps3.tile([1, 512], F32, tag="prow", bufs=3,
                                       name=f"pgx{tagp}")
                        for k in range(NKT):
                            nc.tensor.matmul(pgx[:],
                                             consts[:, cgw + k:cgw + k + 1],
                                             xt[:, k],
                                             start=(k == 0),
                                             stop=(k == NKT - 1))
                    if pwx is None:
                        pwx = ps3.tile([1, 512], F32, tag="prow", bufs=3,
                                       name=f"pwx{tagp}")
                        for k in range(NKT):
                            nc.tensor.matmul(pwx[:],
                                             consts[:, cwv + k:cwv + k + 1],
                                             xt[:, k],
                                             start=(k == 0),
                                             stop=(k == NKT - 1))
                    cs = rows_tile(f"cs{tagp}")
                    nc.scalar.activation(cs[:], phh[:], AF.Abs_reciprocal_sqrt,
                                         scale=1024.0,
                                         bias=eps[0:1, E_CS:E_CS + 1])
                    vg = rows_tile(f"vg{tagp}")
                    nc.scalar.activation(vg[:], pwx[:], AF.Sigmoid)
                    lg = rows_tile(f"lg{tagp}")
                    nc.vector.tensor_mul(lg[:], pgx[:], r_row[:])
                    sg = rows_tile(f"sg{tagp}")
                    nc.scalar.activation(sg[:], lg[:], AF.Sigmoid,
                                         bias=eps[0:1, e_gb:e_gb + 1])
                    kx = rows_tile(f"kx{tagp}")
                    nc.vector.tensor_mul(kx[:], phx[:], cs[:])
                    t2 = rows_tile(f"t2{tagp}")
                    nc.vector.tensor_sub(t2[:], vg[:], kx[:])
                    t3 = rows_tile(f"t3{tagp}")
                    nc.vector.tensor_mul(t3[:], t2[:], sg[:])
                    t4 = rows_tile(f"t4{tagp}")
                    nc.vector.tensor_mul(t4[:], t3[:], cs[:])
                    coef = rows_tile(f"coef{tagp}")
                    nc.vector.tensor_scalar_mul(coef[:], t4[:], 2.0)
                    bco = p3c.tile([128, 512], F32, tag="bco", bufs=1,
                                   name=f"bco{tagp}")
                    nc.gpsimd.partition_broadcast(bco[:], coef[:])
                    for k in range(NKT):
                        tm = p3c.tile([128, 512], F32, tag="dt", bufs=3,
                                      name=f"tm{tagp}{k}")
                        nc.vector.tensor_mul(tm[:], bco[:], ht[:, k])
                        nc.vector.tensor_add(out_t[:, k], tm[:],
                                             xt[:, k].bitcast(F32))

                with tc.tile_pool(name="p3a", bufs=1) as p3a:
                    s2 = p3a.tile([16, 512], F32)
                    nc.vector.tensor_mul(s2[:], sall[:], sall[:])
                    rT = p3a.tile([16, 512], F32R)
                    nc.scalar.activation(rT[:], s2[:], AF.Abs_reciprocal_sqrt,
                                         scale=1.0,
                                         bias=eps[0:16, E_RT:E_RT + 1])
                    aT = p3a.tile([128, NKT, 512], F32R)
                    for kt in range(NKT):
                        pbc = ps3.tile([128, 512], F32, tag="pbig", bufs=4,
                                       name=f"pbc{kt}")
                        nc.tensor.matmul(
                            pbc[:],
                            consts[0:16, C_SEL + 128 * kt:C_SEL + 128 * (kt + 1)],
                            rT[:], start=True, stop=True)
                        nc.vector.tensor_mul(aT[:, kt], araw[:, kt], pbc[:])

                    for m in range(NKT):
                        wot = p3a.tile([128, NKT, 128], F32R, tag="wot",
                                       bufs=3, name=f"wot{m}")
                        nc.sync.dma_start(
                            out=wot[:],
                            in_=r3(wo_in)[:, :, 128 * m:128 * m + 128])
                        po = ps3.tile([128, 512], F32, tag="pbig", bufs=4,
                                      name=f"po{m}")
                        for k in range(NKT):
                            nc.tensor.matmul(po[:], wot[:, k], aT[:, k],
                                             start=(k == 0),
                                             stop=(k == NKT - 1))
                        nc.vector.tensor_copy(hat[:, m], po[:])

                    delta(xown, hat, x1, C_GW1, C_WV1, E_G1B, r1, "d1",
                          pgx=pgx1, pwx=pwx1)

                # r2 + xm
                ps1 = ps3.tile([1, 512], F32, tag="prow", bufs=3, name="ps1")
                for k in range(NKT):
                    xq = p3c.tile([128, 512], F32R, tag="dt", bufs=3,
                                  name=f"x1sq{k}")
                    nc.gpsimd.tensor_mul(xq[:], x1[:, k].bitcast(F32),
                                         x1[:, k].bitcast(F32))
                    nc.tensor.matmul(ps1[:], ONES, xq[:],
                                     start=(k == 0), stop=(k == NKT - 1))
                nc.scalar.activation(r2[:], ps1[:], AF.Abs_reciprocal_sqrt,
                                     scale=1.0 / D,
                                     bias=eps[0:1, E_1EM6:E_1EM6 + 1])
                br2 = p3c.tile([128, 512], F32, tag="bco", bufs=1, name="br2")
                nc.gpsimd.partition_broadcast(br2[:], r2[:])
                for k in range(NKT):
                    nc.vector.tensor_mul(xm[:, k], x1[:, k].bitcast(F32),
                                         br2[:])

                # MLP
                with tc.tile_pool(name="p3b", bufs=1) as p3b:
                    for m in range(NFT):
                        w1t = p3b.tile([128, NKT, 128], F32R, tag="w1t",
                                       bufs=2, name=f"w1t{m}")
                        nc.sync.dma_start(
                            out=w1t[:],
                            in_=r3(w1_in)[:, :, 128 * m:128 * m + 128])
                        ph1 = ps3.tile([128, 512], F32, tag="pbig", bufs=4,
                                       name=f"ph1_{m}")
                        for k in range(NKT):
                            nc.tensor.matmul(ph1[:], w1t[:, k], xm[:, k],
                                             start=(k == 0),
                                             stop=(k == NKT - 1))
                        gs = p3b.tile([128, 512], F32R, tag="gs", bufs=3,
                                      name=f"gs{m}")
                        nc.scalar.activation(gs[:], ph1[:], AF.Silu)
                        w3t = p3b.tile([128, NKT, 128], F32R, tag="w3t",
                                       bufs=2, name=f"w3t{m}")
                        nc.sync.dma_start(
                            out=w3t[:],
                            in_=r3(w3_in)[:, :, 128 * m:128 * m + 128])
                        ph3 = ps3.tile([128, 512], F32, tag="pbig", bufs=4,
                                       name=f"ph3_{m}")
                        for k in range(NKT):
                            nc.tensor.matmul(ph3[:], w3t[:, k], xm[:, k],
                                             start=(k == 0),
                                             stop=(k == NKT - 1))
                        nc.vector.tensor_mul(gt[:, m], gs[:].bitcast(F32),
                                             ph3[:])

                    hmlp = p3c.tile([128, NKT, 512], F32, tag="hbuf",
                                    name="hmlp")
                    for m2 in range(NKT):
                        w2t = p3b.tile([128, NFT, 128], BF16, tag="w2t",
                                       bufs=2, name=f"w2t{m2}")
                        nc.sync.dma_start(
                            out=w2t[:],
                            in_=w2_in[:].rearrange("(f p) t -> p f t", p=128)[
                                :, :, 128 * m2:128 * m2 + 128])
                        py = ps3.tile([128, 512], F32, tag="pbig", bufs=4,
                                      name=f"py{m2}")
                        for f in range(NFT):
                            nc.tensor.matmul(py[:], w2t[:, f], gt[:, f],
                                             start=(f == 0),
                                             stop=(f == NFT - 1))
                        nc.vector.tensor_copy(hmlp[:, m2], py[:])

                    x2 = p3c.tile([128, NKT, 512], F32, tag="xmx2",
                                  name="x2")
                    delta(x1, hmlp, x2, C_GW2, C_WV2, E_G2B, r2, "d2")
                    nc.sync.dma_start(out=r3(out_d), in_=x2[:])

    nc.compile()
    return nc


def _host_prep(inputs):
    f = {k: np.ascontiguousarray(np.asarray(v, np.float32))
         for k, v in inputs.items()}
    anw, mnw = f["attn_norm_w"], f["mlp_norm_w"]
    qn, kn = f["qn_w"], f["kn_w"]
    assert np.allclose(qn[:32], qn[32:]) and np.allclose(qn, kn), \
        "kernel assumes qn_w/kn_w with equal halves (rope-foldable)"
    dd = np.arange(32)
    inv_freq = 1.0 / (10000.0 ** (np.arange(0, HD, 2) / HD))
    t = np.arange(T)
    cos = np.cos(t[None, :] * inv_freq[:, None]).astype(np.float32)
    sin = np.sin(t[None, :] * inv_freq[:, None]).astype(np.float32)
    ctab = np.tile(cos * qn[:32, None], (4, 1)).astype(np.float32)
    stab = np.tile(sin * qn[:32, None], (4, 1)).astype(np.float32)
    tabs = np.concatenate([ctab, stab], axis=1)

    # consts blob
    consts = np.zeros((128, CCOLS), np.float32)
    consts[:, C_I:C_I + 128] = np.eye(128)
    p = np.arange(128)
    h4x32 = (p[:, None] // 32 == np.arange(128)[None, :] // 32).astype(np.float32)
    consts[:, C_H4X32:C_H4X32 + 128] = h4x32
    consts[:, C_H4:C_H4 + 4] = (p[:, None] // 32 == np.arange(4)[None, :])
    consts[:, C_ONES] = 1.0
    consts[:, C_ONES8:C_ONES8 + 8] = 1.0
    consts[:, C_H4P8:C_H4P8 + 4] = (p[:, None] // 32 == np.arange(4)[None, :])
    i_ = np.arange(128)[:, None]
    jj = np.arange(512)[None, :]
    for dk in range(4):
        consts[:, C_MASK + 512 * dk:C_MASK + 512 * (dk + 1)] = np.where(
            jj >= i_ + 128 * dk, 0.0, MASKVAL)
    gwf1 = (f["g1_norm_w"] * f["g1_w"]).reshape(NKT, 128).T
    wv1 = f["wv1"].reshape(NKT, 128).T
    gwf2 = (f["g2_norm_w"] * f["g2_w"]).reshape(NKT, 128).T
    wv2 = f["wv2"].reshape(NKT, 128).T
    consts[:, C_GW1:C_GW1 + 8] = gwf1
    consts[:, C_WV1:C_WV1 + 8] = wv1
    consts[:, C_GW2:C_GW2 + 8] = gwf2
    consts[:, C_WV2:C_WV2 + 8] = wv2
    for kt in range(NKT):
        sel = np.zeros((128, 128), np.float32)
        mm = np.arange(128)
        sel[2 * kt, mm < 64] = 1.0
        sel[2 * kt + 1, mm >= 64] = 1.0
        consts[:, C_SEL + 128 * kt:C_SEL + 128 * (kt + 1)] = sel

    eps = np.zeros((128, 8), np.float32)
    eps[:, E_1EM6] = 1e-6
    eps[:, E_RK] = HD * 1e-6
    eps[:, E_CS] = 1e-9
    eps[:, E_RT] = 1e-30
    eps[:, E_G1B] = float(np.ravel(f["g1_b"])[0])
    eps[:, E_G2B] = float(np.ravel(f["g2_b"])[0])
    eps[:, E_TWO] = 2.0

    wq_f = f["wq"] * anw[None, :]
    wk_f = f["wk"] * anw[None, :]
    wv_f = f["wv"] * anw[None, :]
    w1p = np.zeros((FHP, D), np.float32)
    w1p[:FH] = f["w1"] * mnw[None, :]
    w3p = np.zeros((FHP, D), np.float32)
    w3p[:FH] = f["w3"] * mnw[None, :]
    w2p = np.zeros((FHP, D), np.float32)
    w2p[:FH] = f["w2"].T          # w2_sb = w2.T padded: [FHP, D]
    w1_sb = np.ascontiguousarray(w1p.T)   # [D, FHP]
    w3_sb = np.ascontiguousarray(w3p.T)
    w2_sb = np.ascontiguousarray(w2p).astype(ml_dtypes.bfloat16)
    wo_sb = np.ascontiguousarray(f["wo"].T)

    in_maps = []
    for g in range(NC):
        b, j = g // 4, g % 4
        heads = np.arange(4 * j, 4 * j + 4)
        rows_x1 = (heads[:, None] * HD + dd[None, :]).ravel()
        rows_x2 = (heads[:, None] * HD + 32 + dd[None, :]).ravel()
        rows_split = np.concatenate([rows_x1, rows_x2])
        rows_nat = (heads[:, None] * HD + np.arange(HD)[None, :]).ravel()
        own_cols = np.concatenate(
            [np.arange(64 * (8 * c + g), 64 * (8 * c + g) + 64)
             for c in range(4)])
        x_own = np.concatenate(
            [f["x"][0].T[:, own_cols], f["x"][1].T[:, own_cols]], axis=1)
        in_maps.append({
            "xT": np.ascontiguousarray(f["x"][b].T),
            "x_own": np.ascontiguousarray(x_own),
            "wq_sb": np.ascontiguousarray(wq_f[rows_split].T),
            "wk_sb": np.ascontiguousarray(wk_f[rows_split].T),
            "wv_sb": np.ascontiguousarray(wv_f[rows_nat].T),
            "wo_sb": wo_sb,
            "w1_sb": w1_sb,
            "w3_sb": w3_sb,
            "w2_sb": w2_sb,
            "consts": consts,
            "tabs": tabs,
            "eps": eps,
        })
    return in_maps


def kernel(**inputs):
    if "nc" not in _CACHE:
        _CACHE["nc"] = _build_nc()
    nc = _CACHE["nc"]
    in_maps = _host_prep(inputs)
    res = run_bass_kernel_spmd(nc, in_maps, list(range(NC)))
    _CACHE["last_results"] = res
    out = np.zeros((B, T, D), np.float32)
    for g in range(NC):
        xo = res.results[g]["x_out"]          # [D, 512]
        own = np.concatenate(
            [np.arange(64 * (8 * c + g), 64 * (8 * c + g) + 64)
             for c in range(4)])
        out[0, own, :] = xo[:, 0:256].T
        out[1, own, :] = xo[:, 256:512].T
    return out


# revision 35
# speedup vs baseline: 17535.0319x; 1.0086x over previous
"""Trainium2 Bass kernel for nn_DDLTransformerBlock (8 NeuronCores).

Sharding: core g in 0..7 -> batch b=g//4, heads [4*(g%4), 4*(g%4)+4) for the
attention part (feature-major, fp32r matmuls, split-x1/x2 rope layout, rk
folded into the exp scale); one 8-core AllToAll redistributes attention
output head-blocks to token owners (each core owns a 256-token slice of
EACH batch); O-projection, delta-residual, SwiGLU MLP and the second delta
run token-sharded with fully replicated weights.
"""
import ml_dtypes
import numpy as np
import concourse.bacc as bacc
import concourse.mybir as mybir
from concourse.tile import TileContext
from concourse.bass_utils import run_bass_kernel_spmd

B, T, D, H, HD, FH = 2, 2048, 1024, 16, 64, 2752
FHP = 2816           # FH padded to 22*128
NKT = D // 128       # 8 k-tiles
NFT = FHP // 128     # 22
NC = 8
MASKVAL = -1e6
F32 = mybir.dt.float32
F32R = mybir.dt.float32r
BF16 = mybir.dt.bfloat16
AF = mybir.ActivationFunctionType

# consts blob column layout (fp32r, [128, CCOLS])
C_I = 0            # identity 128
C_H4X32 = 128      # 128
C_H4 = 256         # 4
C_ONES = 260       # 1
C_MASK = 261       # 4*512
C_GW1 = C_MASK + 4 * 512   # 8
C_WV1 = C_GW1 + 8          # 8
C_GW2 = C_WV1 + 8          # 8
C_WV2 = C_GW2 + 8          # 8
C_SEL = C_WV2 + 8          # 8*128 (sel_kt: [16,128] blocks)
C_ONES8 = C_SEL + 8 * 128  # 8 (all-ones cols)
C_H4P8 = C_ONES8 + 8       # 8 (H4 padded to 8 cols)
CCOLS = C_H4P8 + 8
# eps tile columns (f32, [128, 8])
E_1EM6, E_RK, E_CS, E_RT, E_G1B, E_G2B, E_ZERO, E_TWO = range(8)

_CACHE = {}


def _build_nc():
    nc = bacc.Bacc("TRN2", target_bir_lowering=False, num_devices=NC)
    xT_in = nc.declare_dram_parameter("xT", [D, T], F32R, isOutput=False)
    xo_in = nc.declare_dram_parameter("x_own", [D, 512], F32R, isOutput=False)
    wq_in = nc.declare_dram_parameter("wq_sb", [D, 256], F32R, isOutput=False)
    wk_in = nc.declare_dram_parameter("wk_sb", [D, 256], F32R, isOutput=False)
    wv_in = nc.declare_dram_parameter("wv_sb", [D, 256], F32R, isOutput=False)
    wo_in = nc.declare_dram_parameter("wo_sb", [D, D], F32R, isOutput=False)
    w1_in = nc.declare_dram_parameter("w1_sb", [D, FHP], F32R, isOutput=False)
    w3_in = nc.declare_dram_parameter("w3_sb", [D, FHP], F32R, isOutput=False)
    w2_in = nc.declare_dram_parameter("w2_sb", [FHP, D], BF16, isOutput=False)
    cn_in = nc.declare_dram_parameter("consts", [128, CCOLS], F32R, isOutput=False)
    tb_in = nc.declare_dram_parameter("tabs", [128, 2 * T], F32, isOutput=False)
    ep_in = nc.declare_dram_parameter("eps", [128, 8], F32, isOutput=False)
    out_d = nc.declare_dram_parameter("x_out", [D, 512], F32, isOutput=True)

    def r3(dram, p=128):
        # view [R, C] dram as (p, ktile, C) for tile DMAs
        return dram[:].rearrange("(k p) t -> p k t", p=p)

    with TileContext(nc) as tc:
        with tc.tile_pool(name="glob", bufs=1) as glob, \
             tc.tile_pool(name="dram", bufs=1, space="DRAM") as dram:
            consts = glob.tile([128, CCOLS], F32R)
            nc.sync.dma_start(out=consts[:], in_=cn_in[:])
            eps = glob.tile([128, 8], F32)
            nc.sync.dma_start(out=eps[:], in_=ep_in[:])
            xown = glob.tile([128, NKT, 512], F32R)

            I128 = consts[:, C_I:C_I + 128]
            H4X32 = consts[:, C_H4X32:C_H4X32 + 128]
            H4 = consts[:, C_H4:C_H4 + 4]
            ONES8 = consts[:, C_ONES8:C_ONES8 + 8]
            H4P8 = consts[:, C_H4P8:C_H4P8 + 8]
            ONES = consts[:, C_ONES:C_ONES + 1]
            masks = [consts[:, C_MASK + 512 * i: C_MASK + 512 * (i + 1)]
                     for i in range(4)]

            SHW = 128 * 2 * 64 + 4 * 64   # shard words: features + sums
            bounce_in = [dram.tile([NC, SHW], F32, name=f"bin{c}")
                         for c in range(4)]
            bounce_out = [dram.tile([NC, SHW], F32, name=f"bout{c}")
                          for c in range(4)]
            araw = glob.tile([128, NKT, 512], F32)
            sall = glob.tile([16, 512], F32)

            # ============ PHASE 1+2: QKV + attention (per tq-chunk) ========
            with tc.tile_pool(name="p12", bufs=1) as p12, \
                 tc.tile_pool(name="ps12", bufs=1, space="PSUM") as ps12:
                tabs = p12.tile([128, 2 * T], F32)
                nc.sync.dma_start(out=tabs[:], in_=tb_in[:])
                ctab = tabs[:, 0:T]
                stab = tabs[:, T:2 * T]
                wq = p12.tile([128, NKT, 256], F32R)
                nc.sync.dma_start(out=wq[:], in_=r3(wq_in))
                wk = p12.tile([128, NKT, 256], F32R)
                nc.sync.dma_start(out=wk[:], in_=r3(wk_in))
                wv = p12.tile([128, NKT, 256], F32R)
                nc.sync.dma_start(out=wv[:], in_=r3(wv_in))

                qpe1 = p12.tile([128, T], F32R)
                qpe2 = p12.tile([128, T], F32R)
                kpe1 = p12.tile([128, T], F32R)
                kpe2 = p12.tile([128, T], F32R)
                v4 = p12.tile([128, 16, 4, 65], F32R)
                nc.gpsimd.memset(v4[:, :, :, 64:65].bitcast(F32), 1.0)
                rk_c = p12.tile([128, 16, 4], F32)
                rv_c = p12.tile([128, 16], F32)

                for c in range(4):
                    ts = slice(512 * c, 512 * c + 512)
                    xc = p12.tile([128, NKT, 512], F32R, tag="xc", bufs=2,
                                  name=f"xc{c}")
                    nc.sync.dma_start(out=xc[:], in_=r3(xT_in)[:, :, ts])
                    # squares of x (for rv), rotating per k-tile
                    prv = ps12.tile([128, 32], F32, tag="psx", bufs=3,
                                    name=f"prv{c}")
                    for k in range(NKT):
                        xsq = p12.tile([128, 512], F32R, tag="xsq", bufs=2,
                                       name=f"xsq{c}_{k}")
                        nc.gpsimd.tensor_mul(xsq[:], xc[:, k].bitcast(F32),
                                             xc[:, k].bitcast(F32))
                        for tt in range(4):
                            nc.tensor.matmul(
                                prv[:, 8 * tt:8 * tt + 8],
                                xsq[:, 128 * tt:128 * tt + 128], ONES8,
                                start=(k == 0), stop=(k == NKT - 1))
                    nc.scalar.activation(
                        rv_c[:, 4 * c:4 * c + 4],
                        prv[:].rearrange("p (t e) -> p t e", e=8)[:, :, 0],
                        AF.Abs_reciprocal_sqrt,
                        scale=1.0 / D, bias=eps[:, E_1EM6:E_1EM6 + 1])

                    # q projection (2 m-tiles) + per-head rsqrt + rope
                    pq = [ps12.tile([128, 512], F32, tag="pqk", bufs=2,
                                    name=f"pq{c}_{m}") for m in range(2)]
                    for m in range(2):
                        for k in range(NKT):
                            nc.tensor.matmul(
                                pq[m][:], wq[:, k, 128 * m:128 * m + 128],
                                xc[:, k], start=(k == 0), stop=(k == NKT - 1))
                    qsq = [p12.tile([128, 512], F32R, tag="qsq", bufs=4,
                                    name=f"qsq{c}_{m}") for m in range(2)]
                    for m in range(2):
                        nc.scalar.square(qsq[m][:], pq[m][:])
                    pssq = ps12.tile([128, 512], F32, tag="psx", bufs=3,
                                     name=f"pssq{c}")
                    nc.tensor.matmul(pssq[:], H4X32, qsq[0][:],
                                     start=True, stop=False)
                    nc.tensor.matmul(pssq[:], H4X32, qsq[1][:],
                                     start=False, stop=True)
                    bq = p12.tile([128, 512], F32, tag="bq", bufs=2,
                                  name=f"bq{c}")
                    nc.scalar.activation(
                        bq[:], pssq[:], AF.Abs_reciprocal_sqrt,
                        scale=1.0 / HD, bias=eps[:, E_1EM6:E_1EM6 + 1])
                    ra = p12.tile([128, 512], F32, tag="rt", bufs=3,
                                  name=f"ra{c}")
                    rb = p12.tile([128, 512], F32, tag="rt", bufs=3,
                                  name=f"rb{c}")
                    ro = p12.tile([128, 512], F32, tag="rt", bufs=3,
                                  name=f"ro{c}")
                    nc.vector.tensor_mul(ra[:], pq[0][:], ctab[:, ts])
                    nc.vector.tensor_mul(rb[:], pq[1][:], stab[:, ts])
                    nc.vector.tensor_sub(ro[:], ra[:], rb[:])
                    nc.vector.tensor_mul(qpe1[:, ts], ro[:], bq[:])
                    rc = p12.tile([128, 512], F32, tag="rt", bufs=3,
                                  name=f"rc{c}")
                    rd = p12.tile([128, 512], F32, tag="rt", bufs=3,
                                  name=f"rd{c}")
                    ro2 = p12.tile([128, 512], F32, tag="rt", bufs=3,
                                   name=f"ro2{c}")
                    nc.vector.tensor_mul(rc[:], pq[1][:], ctab[:, ts])
                    nc.vector.tensor_mul(rd[:], pq[0][:], stab[:, ts])
                    nc.vector.tensor_add(ro2[:], rc[:], rd[:])
                    nc.vector.tensor_mul(qpe2[:, ts], ro2[:], bq[:])

                    # k projection + rk cols + rope (unnormalized)
                    pk = [ps12.tile([128, 512], F32, tag="pqk", bufs=2,
                                    name=f"pk{c}_{m}") for m in range(2)]
                    for m in range(2):
                        for k in range(NKT):
                            nc.tensor.matmul(
                                pk[m][:], wk[:, k, 128 * m:128 * m + 128],
                                xc[:, k], start=(k == 0), stop=(k == NKT - 1))
                    ksq = [p12.tile([128, 512], F32R, tag="qsq", bufs=4,
                                    name=f"ksq{c}_{m}") for m in range(2)]
                    for m in range(2):
                        nc.scalar.square(ksq[m][:], pk[m][:])
                    prk = ps12.tile([128, 32], F32, tag="psx", bufs=3,
                                    name=f"prk{c}")
                    for tt in range(4):
                        nc.tensor.matmul(
                            prk[:, 8 * tt:8 * tt + 8],
                            ksq[0][:, 128 * tt:128 * tt + 128], H4P8,
                            start=True, stop=False)
                        nc.tensor.matmul(
                            prk[:, 8 * tt:8 * tt + 8],
                            ksq[1][:, 128 * tt:128 * tt + 128], H4P8,
                            start=False, stop=True)
                    nc.scalar.activation(
                        rk_c[:, 4 * c:4 * c + 4, :],
                        prk[:].rearrange("p (t h) -> p t h", h=8)[:, :, 0:4],
                        AF.Abs_reciprocal_sqrt,
                        scale=1.0, bias=eps[:, E_RK:E_RK + 1])
                    ka = p12.tile([128, 512], F32, tag="rt", bufs=3,
                                  name=f"ka{c}")
                    kb = p12.tile([128, 512], F32, tag="rt", bufs=3,
                                  name=f"kb{c}")
                    nc.vector.tensor_mul(ka[:], pk[0][:], ctab[:, ts])
                    nc.vector.tensor_mul(kb[:], pk[1][:], stab[:, ts])
                    nc.vector.tensor_sub(kpe1[:, ts], ka[:], kb[:])
                    kc = p12.tile([128, 512], F32, tag="rt", bufs=3,
                                  name=f"kc{c}")
                    kd = p12.tile([128, 512], F32, tag="rt", bufs=3,
                                  name=f"kd{c}")
                    nc.vector.tensor_mul(kc[:], pk[1][:], ctab[:, ts])
                    nc.vector.tensor_mul(kd[:], pk[0][:], stab[:, ts])
                    nc.vector.tensor_add(kpe2[:, ts], kc[:], kd[:])

                    # v projection, scaled by rv on psum->sbuf copy
                    for tt in range(4):
                        pv = ps12.tile([128, 256], F32, tag="pqk", bufs=2,
                                       name=f"pv{c}_{tt}")
                        for k in range(NKT):
                            nc.tensor.matmul(
                                pv[:], xc[:, k, 128 * tt:128 * tt + 128],
                                wv[:, k], start=(k == 0), stop=(k == NKT - 1))
                        nc.vector.tensor_scalar_mul(
                            v4[:, 4 * c + tt, :, 0:64],
                            pv[:].rearrange("p (h d) -> p h d", d=64),
                            rv_c[:, 4 * c + tt:4 * c + tt + 1])

                    # ---- attention for tq-chunk c, head pairs {0,1}, {2,3}
                    njt = 4 * (c + 1)
                    for hp in range(2):
                        pO = [ps12.tile([65, 512], F32, tag="pO", bufs=2,
                                        name=f"pO{c}_{hp}_{hh}")
                              for hh in range(2)]
                        for j in range(njt):
                            ks = slice(128 * j, 128 * j + 128)
                            diag = j >= 4 * c
                            for hh in range(2):
                                h = 2 * hp + hh
                                hs = slice(32 * h, 32 * h + 32)
                                pS = ps12.tile([128, 512], F32, tag="psx",
                                               bufs=3, name=f"pS{c}_{hp}_{j}_{hh}")
                                if diag:
                                    nc.tensor.matmul(pS[:], I128,
                                                     masks[j - 4 * c],
                                                     start=True, stop=False)
                                nc.tensor.matmul(
                                    pS[:], kpe1[hs, ks], qpe1[hs, ts],
                                    start=not diag, stop=False,
                                    tile_position=(32 * h, 0))
                                nc.tensor.matmul(
                                    pS[:], kpe2[hs, ks], qpe2[hs, ts],
                                    start=False, stop=True,
                                    tile_position=(32 * h, 0))
                                pT = p12.tile([128, 512], F32R, tag="pT",
                                              bufs=3, name=f"pT{c}_{hp}_{j}_{hh}")
                                nc.scalar.activation(
                                    pT[:], pS[:], AF.Exp,
                                    scale=rk_c[:, j, h:h + 1])
                                nc.tensor.matmul(
                                    pO[hh][:], v4[:, j, h, :], pT[:],
                                    start=(j == 0), stop=(j == njt - 1))
                        for hh in range(2):
                            h = 2 * hp + hh
                            st = p12.tile([65, 512], F32, tag="st", bufs=3,
                                          name=f"st{c}_{hp}_{hh}")
                            nc.vector.tensor_copy(st[:], pO[hh][:])
                            qv = bounce_in[c][:].rearrange(
                                "s (q t) -> s q t", t=64)
                            fview = qv[:, 0:256, :].rearrange(
                                "s (p k) t -> s p k t", k=2)
                            nc.sync.dma_start(
                                out=fview[:, 64 * (h % 2):64 * (h % 2) + 64,
                                          h // 2, :].transpose([1, 0, 2]),
                                in_=st[0:64, :].rearrange(
                                    "p (s t) -> p s t", t=64))
                            nc.sync.dma_start(
                                out=qv[:, 256 + h, :].unsqueeze(0),
                                in_=st[64:65, :].rearrange(
                                    "p (s t) -> p s t", t=64))
                    nc.gpsimd.collective_compute(
                        "AllToAll", mybir.AluOpType.bypass,
                        replica_groups=[list(range(NC))],
                        ins=[bounce_in[c][:]], outs=[bounce_out[c][:]],
                    )
                    # receive chunk-c shards (overlaps later chunks)
                    for i in range(NC):
                        bi, m = i // 4, i % 4
                        col = 256 * bi + 64 * c
                        qview = bounce_out[c][:].rearrange(
                            "s (q t) -> s q t", t=64)
                        nc.sync.dma_start(
                            out=araw[:, 2 * m:2 * m + 2, col:col + 64],
                            in_=qview[i, 0:256, :].rearrange(
                                "(p k) t -> p k t", k=2))
                        nc.sync.dma_start(
                            out=sall[4 * m:4 * m + 4, col:col + 64],
                            in_=qview[i, 256:260, :])

            # ============ PHASE 3: O-proj + delta1 + MLP + delta2 ==========
            with tc.tile_pool(name="p3c", bufs=1) as p3c, \
                 tc.tile_pool(name="ps3", bufs=1, space="PSUM") as ps3:
                nc.sync.dma_start(out=xown[:], in_=r3(xo_in))
                x1 = p3c.tile([128, NKT, 512], F32R)
                xm = p3c.tile([128, NKT, 512], F32R, tag="xmx2")
                hat = p3c.tile([128, NKT, 512], F32, tag="hbuf")
                gt = p3c.tile([128, NFT, 512], BF16)
                r2 = p3c.tile([1, 512], F32)

                def rows_tile(nm):
                    return p3c.tile([1, 512], F32, tag="rows", bufs=5, name=nm)

                # delta1 reductions that depend only on x_own: emit early so
                # PE fills the tail of the attention/A2A window
                pso = ps3.tile([1, 512], F32, tag="prow", bufs=3, name="pso")
                for k in range(NKT):
                    xq = p3c.tile([128, 512], F32R, tag="dt", bufs=3,
                                  name=f"xosq{k}")
                    nc.gpsimd.tensor_mul(xq[:], xown[:, k].bitcast(F32),
                                         xown[:, k].bitcast(F32))
                    nc.tensor.matmul(pso[:], ONES, xq[:],
                                     start=(k == 0), stop=(k == NKT - 1))
                r1 = p3c.tile([1, 512], F32, name="r1")
                nc.scalar.activation(r1[:], pso[:], AF.Abs_reciprocal_sqrt,
                                     scale=1.0 / D,
                                     bias=eps[0:1, E_1EM6:E_1EM6 + 1])
                pgx1 = ps3.tile([1, 512], F32, tag="prow", bufs=3,
                                name="pgx1")
                for k in range(NKT):
                    nc.tensor.matmul(pgx1[:],
                                     consts[:, C_GW1 + k:C_GW1 + k + 1],
                                     xown[:, k],
                                     start=(k == 0), stop=(k == NKT - 1))
                pwx1 = ps3.tile([1, 512], F32, tag="prow", bufs=3,
                                name="pwx1")
                for k in range(NKT):
                    nc.tensor.matmul(pwx1[:],
                                     consts[:, C_WV1 + k:C_WV1 + k + 1],
                                     xown[:, k],
                                     start=(k == 0), stop=(k == NKT - 1))

                def delta(xt, ht, out_t, cgw, cwv, e_gb, r_row, tagp,
                          pgx=None, pwx=None):
                    phh = ps3.tile([1, 512], F32, tag="prow", bufs=3,
                                   name=f"phh{tagp}")
                    phx = ps3.tile([1, 512], F32, tag="prow", bufs=3,
                                   name=f"phx{tagp}")
                    for k in range(NKT):
                        hsq = p3c.tile([128, 512], F32R, tag="dt", bufs=3,
                                       name=f"hsq{tagp}{k}")
                        nc.gpsimd.tensor_mul(hsq[:], ht[:, k], ht[:, k])
                        nc.tensor.matmul(phh[:], ONES, hsq[:],
                                         start=(k == 0), stop=(k == NKT - 1))
                    for k in range(NKT):
                        hx = p3c.tile([128, 512], F32R, tag="dt", bufs=3,
                                      name=f"hx{tagp}{k}")
                        nc.gpsimd.tensor_mul(hx[:], ht[:, k],
                                             xt[:, k].bitcast(F32))
                        nc.tensor.matmul(phx[:], ONES, hx[:],
                                         start=(k == 0), stop=(k == NKT - 1))
                    if pgx is None:
                        pgx = ps3.tile([1, 512], F32, tag="prow", bufs=3,
                                       name=f"pgx{tagp}")
                        for k in range(NKT):
                            nc.tensor.matmul(pgx[:],
                                             consts[:, cgw + k:cgw + k + 1],
                                             xt[:, k],
                                             start=(k == 0),
                                             stop=(k == NKT - 1))
                    if pwx is None:
                        pwx = ps3.tile([1, 512], F32, tag="prow", bufs=3,
                                       name=f"pwx{tagp}")
                        for k in range(NKT):
                            nc.tensor.matmul(pwx[:],
                                             consts[:, cwv + k:cwv + k + 1],
                                             xt[:, k],
                                             start=(k == 0),
                                             stop=(k == NKT - 1))
                    cs = rows_tile(f"cs{tagp}")
                    nc.scalar.activation(cs[:], phh[:], AF.Abs_reciprocal_sqrt,
                                         scale=1024.0,
                                         bias=eps[0:1, E_CS:E_CS + 1])
                    vg = rows_tile(f"vg{tagp}")
                    nc.scalar.activation(vg[:], pwx[:], AF.Sigmoid)
                    lg = rows_tile(f"lg{tagp}")
                    nc.vector.tensor_mul(lg[:], pgx[:], r_row[:])
                    sg = rows_tile(f"sg{tagp}")
                    nc.scalar.activation(sg[:], lg[:], AF.Sigmoid,
                                         bias=eps[0:1, e_gb:e_gb + 1])
                    kx = rows_tile(f"kx{tagp}")
                    nc.vector.tensor_mul(kx[:], phx[:], cs[:])
                    t2 = rows_tile(f"t2{tagp}")
                    nc.vector.tensor_sub(t2[:], vg[:], kx[:])
                    t3 = rows_tile(f"t3{tagp}")
                    nc.vector.tensor_mul(t3[:], t2[:], sg[:])
                    t4 = rows_tile(f"t4{tagp}")
                    nc.vector.tensor_mul(t4[:], t3[:], cs[:])
                    coef = rows_tile(f"coef{tagp}")
                    nc.vector.tensor_scalar_mul(coef[:], t4[:], 2.0)
                    bco = p3c.tile([128, 512], F32, tag="bco", bufs=1,
                                   name=f"bco{tagp}")
                    nc.gpsimd.partition_broadcast(bco[:], coef[:])
                    for k in range(NKT):
                        tm = p3c.tile([128, 512], F32, tag="dt", bufs=3,
                                      name=f"tm{tagp}{k}")
                        eng = nc.vector if k % 2 == 0 else nc.gpsimd
                        eng.tensor_mul(tm[:], bco[:], ht[:, k])
                        eng.tensor_add(out_t[:, k], tm[:],
                                       xt[:, k].bitcast(F32))

                with tc.tile_pool(name="p3a", bufs=1) as p3a:
                    s2 = p3a.tile([16, 512], F32)
                    nc.vector.tensor_mul(s2[:], sall[:], sall[:])
                    rT = p3a.tile([16, 512], F32R)
                    nc.scalar.activation(rT[:], s2[:], AF.Abs_reciprocal_sqrt,
                                         scale=1.0,
                                         bias=eps[0:16, E_RT:E_RT + 1])
                    aT = p3a.tile([128, NKT, 512], F32R)
                    for kt in range(NKT):
                        pbc = ps3.tile([128, 512], F32, tag="pbig", bufs=4,
                                       name=f"pbc{kt}")
                        nc.tensor.matmul(
                            pbc[:],
                            consts[0:16, C_SEL + 128 * kt:C_SEL + 128 * (kt + 1)],
                            rT[:], start=True, stop=True)
                        nc.vector.tensor_mul(aT[:, kt], araw[:, kt], pbc[:])

                    for m in range(NKT):
                        wot = p3a.tile([128, NKT, 128], F32R, tag="wot",
                                       bufs=3, name=f"wot{m}")
                        nc.sync.dma_start(
                            out=wot[:],
                            in_=r3(wo_in)[:, :, 128 * m:128 * m + 128])
                        po = ps3.tile([128, 512], F32, tag="pbig", bufs=4,
                                      name=f"po{m}")
                        for k in range(NKT):
                            nc.tensor.matmul(po[:], wot[:, k], aT[:, k],
                                             start=(k == 0),
                                             stop=(k == NKT - 1))
                        nc.vector.tensor_copy(hat[:, m], po[:])

                    delta(xown, hat, x1, C_GW1, C_WV1, E_G1B, r1, "d1",
                          pgx=pgx1, pwx=pwx1)

                # r2 + xm
                ps1 = ps3.tile([1, 512], F32, tag="prow", bufs=3, name="ps1")
                for k in range(NKT):
                    xq = p3c.tile([128, 512], F32R, tag="dt", bufs=3,
                                  name=f"x1sq{k}")
                    nc.gpsimd.tensor_mul(xq[:], x1[:, k].bitcast(F32),
                                         x1[:, k].bitcast(F32))
                    nc.tensor.matmul(ps1[:], ONES, xq[:],
                                     start=(k == 0), stop=(k == NKT - 1))
                nc.scalar.activation(r2[:], ps1[:], AF.Abs_reciprocal_sqrt,
                                     scale=1.0 / D,
                                     bias=eps[0:1, E_1EM6:E_1EM6 + 1])
                br2 = p3c.tile([128, 512], F32, tag="bco", bufs=1, name="br2")
                nc.gpsimd.partition_broadcast(br2[:], r2[:])
                for k in range(NKT):
                    eng = nc.vector if k % 2 == 0 else nc.gpsimd
                    eng.tensor_mul(xm[:, k], x1[:, k].bitcast(F32), br2[:])

                # MLP
                with tc.tile_pool(name="p3b", bufs=1) as p3b:
                    for m in range(NFT):
                        w1t = p3b.tile([128, NKT, 128], F32R, tag="w1t",
                                       bufs=2, name=f"w1t{m}")
                        nc.sync.dma_start(
                            out=w1t[:],
                            in_=r3(w1_in)[:, :, 128 * m:128 * m + 128])
                        ph1 = ps3.tile([128, 512], F32, tag="pbig", bufs=4,
                                       name=f"ph1_{m}")
                        for k in range(NKT):
                            nc.tensor.matmul(ph1[:], w1t[:, k], xm[:, k],
                                             start=(k == 0),
                                             stop=(k == NKT - 1))
                        gs = p3b.tile([128, 512], F32R, tag="gs", bufs=3,
                                      name=f"gs{m}")
                        nc.scalar.activation(gs[:], ph1[:], AF.Silu)
                        w3t = p3b.tile([128, NKT, 128], F32R, tag="w3t",
                                       bufs=2, name=f"w3t{m}")
                        nc.sync.dma_start(
                            out=w3t[:],
                            in_=r3(w3_in)[:, :, 128 * m:128 * m + 128])
                        ph3 = ps3.tile([128, 512], F32, tag="pbig", bufs=4,
                                       name=f"ph3_{m}")
                        for k in range(NKT):
                            nc.tensor.matmul(ph3[:], w3t[:, k], xm[:, k],
                                             start=(k == 0),
                                             stop=(k == NKT - 1))
                        nc.vector.tensor_mul(gt[:, m], gs[:].bitcast(F32),
                                             ph3[:])

                    hmlp = p3c.tile([128, NKT, 512], F32, tag="hbuf",
                                    name="hmlp")
                    for m2 in range(NKT):
                        w2t = p3b.tile([128, NFT, 128], BF16, tag="w2t",
                                       bufs=2, name=f"w2t{m2}")
                        nc.sync.dma_start(
                            out=w2t[:],
                            in_=w2_in[:].rearrange("(f p) t -> p f t", p=128)[
                                :, :, 128 * m2:128 * m2 + 128])
                        py = ps3.tile([128, 512], F32, tag="pbig", bufs=4,
                                      name=f"py{m2}")
                        for f in range(NFT):
                            nc.tensor.matmul(py[:], w2t[:, f], gt[:, f],
                                             start=(f == 0),
                                             stop=(f == NFT - 1))
                        nc.vector.tensor_copy(hmlp[:, m2], py[:])

                    x2 = p3c.tile([128, NKT, 512], F32, tag="xmx2",
                                  name="x2")
                    delta(x1, hmlp, x2, C_GW2, C_WV2, E_G2B, r2, "d2")
                    nc.sync.dma_start(out=r3(out_d), in_=x2[:])

    nc.compile()
    return nc


def _host_prep(inputs):
    f = {k: np.ascontiguousarray(np.asarray(v, np.float32))
         for k, v in inputs.items()}
    anw, mnw = f["attn_norm_w"], f["mlp_norm_w"]
    qn, kn = f["qn_w"], f["kn_w"]
    assert np.allclose(qn[:32], qn[32:]) and np.allclose(qn, kn), \
        "kernel assumes qn_w/kn_w with equal halves (rope-foldable)"
    dd = np.arange(32)
    inv_freq = 1.0 / (10000.0 ** (np.arange(0, HD, 2) / HD))
    t = np.arange(T)
    cos = np.cos(t[None, :] * inv_freq[:, None]).astype(np.float32)
    sin = np.sin(t[None, :] * inv_freq[:, None]).astype(np.float32)
    ctab = np.tile(cos * qn[:32, None], (4, 1)).astype(np.float32)
    stab = np.tile(sin * qn[:32, None], (4, 1)).astype(np.float32)
    tabs = np.concatenate([ctab, stab], axis=1)

    # consts blob
    consts = np.zeros((128, CCOLS), np.float32)
    consts[:, C_I:C_I + 128] = np.eye(128)
    p = np.arange(128)
    h4x32 = (p[:, None] // 32 == np.arange(128)[None, :] // 32).astype(np.float32)
    consts[:, C_H4X32:C_H4X32 + 128] = h4x32
    consts[:, C_H4:C_H4 + 4] = (p[:, None] // 32 == np.arange(4)[None, :])
    consts[:, C_ONES] = 1.0
    consts[:, C_ONES8:C_ONES8 + 8] = 1.0
    consts[:, C_H4P8:C_H4P8 + 4] = (p[:, None] // 32 == np.arange(4)[None, :])
    i_ = np.arange(128)[:, None]
    jj = np.arange(512)[None, :]
    for dk in range(4):
        consts[:, C_MASK + 512 * dk:C_MASK + 512 * (dk + 1)] = np.where(
            jj >= i_ + 128 * dk, 0.0, MASKVAL)
    gwf1 = (f["g1_norm_w"] * f["g1_w"]).reshape(NKT, 128).T
    wv1 = f["wv1"].reshape(NKT, 128).T
    gwf2 = (f["g2_norm_w"] * f["g2_w"]).reshape(NKT, 128).T
    wv2 = f["wv2"].reshape(NKT, 128).T
    consts[:, C_GW1:C_GW1 + 8] = gwf1
    consts[:, C_WV1:C_WV1 + 8] = wv1
    consts[:, C_GW2:C_GW2 + 8] = gwf2
    consts[:, C_WV2:C_WV2 + 8] = wv2
    for kt in range(NKT):
        sel = np.zeros((128, 128), np.float32)
        mm = np.arange(128)
        sel[2 * kt, mm < 64] = 1.0
        sel[2 * kt + 1, mm >= 64] = 1.0
        consts[:, C_SEL + 128 * kt:C_SEL + 128 * (kt + 1)] = sel

    eps = np.zeros((128, 8), np.float32)
    eps[:, E_1EM6] = 1e-6
    eps[:, E_RK] = HD * 1e-6
    eps[:, E_CS] = 1e-9
    eps[:, E_RT] = 1e-30
    eps[:, E_G1B] = float(np.ravel(f["g1_b"])[0])
    eps[:, E_G2B] = float(np.ravel(f["g2_b"])[0])
    eps[:, E_TWO] = 2.0

    wq_f = f["wq"] * anw[None, :]
    wk_f = f["wk"] * anw[None, :]
    wv_f = f["wv"] * anw[None, :]
    w1p = np.zeros((FHP, D), np.float32)
    w1p[:FH] = f["w1"] * mnw[None, :]
    w3p = np.zeros((FHP, D), np.float32)
    w3p[:FH] = f["w3"] * mnw[None, :]
    w2p = np.zeros((FHP, D), np.float32)
    w2p[:FH] = f["w2"].T          # w2_sb = w2.T padded: [FHP, D]
    w1_sb = np.ascontiguousarray(w1p.T)   # [D, FHP]
    w3_sb = np.ascontiguousarray(w3p.T)
    w2_sb = np.ascontiguousarray(w2p).astype(ml_dtypes.bfloat16)
    wo_sb = np.ascontiguousarray(f["wo"].T)

    in_maps = []
    for g in range(NC):
        b, j = g // 4, g % 4
        heads = np.arange(4 * j, 4 * j + 4)
        rows_x1 = (heads[:, None] * HD + dd[None, :]).ravel()
        rows_x2 = (heads[:, None] * HD + 32 + dd[None, :]).ravel()
        rows_split = np.concatenate([rows_x1, rows_x2])
        rows_nat = (heads[:, None] * HD + np.arange(HD)[None, :]).ravel()
        own_cols = np.concatenate(
            [np.arange(64 * (8 * c + g), 64 * (8 * c + g) + 64)
             for c in range(4)])
        x_own = np.concatenate(
            [f["x"][0].T[:, own_cols], f["x"][1].T[:, own_cols]], axis=1)
        in_maps.append({
            "xT": np.ascontiguousarray(f["x"][b].T),
            "x_own": np.ascontiguousarray(x_own),
            "wq_sb": np.ascontiguousarray(wq_f[rows_split].T),
            "wk_sb": np.ascontiguousarray(wk_f[rows_split].T),
            "wv_sb": np.ascontiguousarray(wv_f[rows_nat].T),
            "wo_sb": wo_sb,
            "w1_sb": w1_sb,
            "w3_sb": w3_sb,
            "w2_sb": w2_sb,
            "consts": consts,
            "tabs": tabs,
            "eps": eps,
        })
    return in_maps


def kernel(**inputs):
    if "nc" not in _CACHE:
        _CACHE["nc"] = _build_nc()
    nc = _CACHE["nc"]
    in_maps = _host_prep(inputs)
    res = run_bass_kernel_spmd(nc, in_maps, list(range(NC)))
    _CACHE["last_results"] = res
    out = np.zeros((B, T, D), np.float32)
    for g in range(NC):
        xo = res.results[g]["x_out"]          # [D, 512]
        own = np.concatenate(
            [np.arange(64 * (8 * c + g), 64 * (8 * c + g) + 64)
             for c in range(4)])
        out[0, own, :] = xo[:, 0:256].T
        out[1, own, :] = xo[:, 256:512].T
    return out
